# revision 51
# baseline (speedup 1.0000x reference)
"""BiSPA (bidirectional sparse windowed attention + MLP) Trainium2 kernel.

Full inputs in, full outputs out; core c owns output rows [24c, 24c+24).
Optimized v9 (~458-465us at full clock vs 511us v8 / 1065us baseline;
P0 throttle phases add up to ~10%). Key ideas on top of v8:
- feature-major turn (ctx token-major -> MLP feature-major) moved off the
  PE: XBAR DMA-transposes on the sync queue ([96,1024] -> [128,8,96] into
  a single per-group ct tile), replacing 32 PE transposes + 8 DVE copies
  per group. PSUM freed by pcx -> psS double-buffered.
- softmax normalization as stride-0-broadcast DVE tensor_tensor
  ([96, nh, 64] ctx times recip[96, nh, 1]): 2 ops per attn@V bank
  instead of 2*nh per-head muls split across DVE/ScalarE (ScalarE busy
  68% -> 45%).
- score pairs packed 2-per-[128,1024] tile with h2=0/h2=1 blocks in
  separate banks: quad row-tiles drain to distinct PSUM banks, so each
  bank must stay row-group pure (mixing row groups in a bank hangs the
  NEFF). Frees 2 banks -> psA triple-buffered.
- final group's MLP split per strip (token columns are independent):
  strip-0 half runs while strip-1 attention/transposes finish.
- startup: weights on the sync queue in consumption order (whq, whkv,
  wv-QK | wv-V split), xr before xc; first matmul at ~10us.
- attn@V ones-columns written once (group 0 touches all 8 pv buffers).
- output DMA'd as bf16 (half the bytes), host converts to fp32.
Known limits: scores/attn@V are LDWEIGHTS-port-bound (~80-90ns/matmul vs
40ns stream); fp8 fails the 2e-2 gate (V-proj/MLP2 ~3.4-4.3e-2 alone);
ctx^T attn@V dead-ends on partition-broadcast for Z (DVE lanes are
partition-locked, gpsimd ops cost ~14.5us each).
"""

import numpy as np
from contextlib import ExitStack

import concourse.bass as bass
import concourse.mybir as mybir
import concourse.tile as tile
from concourse import bacc
from concourse.bass_utils import run_bass_kernel_spmd
from concourse.tile import add_dep_helper


def _chain(insts):
    for a, b in zip(insts, insts[1:]):
        add_dep_helper(b.ins, a.ins, sync=False, reason="psum-bank group order")

BF = mybir.dt.bfloat16
F32 = mybir.dt.float32
AF = mybir.ActivationFunctionType
MUL = mybir.AluOpType.mult
NPBF = mybir.dt.np(BF)

E = 512
H = 8
D = 64
W = 32
S = 192
NCORE = 8
RPC = 24
T = RPC * S


def _band_masks():
    """Score mask, bf16 (128, 384): [TA 96 | TB 96] x 2 heads.

    q-blocks split at 96 so neither straddles a key block:
      TA: rows k in [0,128), cols q in [0,96):    valid = |k-q| <= W
      TB: rows k = 64+r in [64,192), cols q = 96+c in [96,192):
          valid = |k-q| <= W
    """
    k = np.arange(128)[:, None]
    qa = np.arange(96)[None, :]
    ta = (np.abs(k - qa) <= W)
    kb = 64 + np.arange(128)[:, None]
    qb = 96 + np.arange(96)[None, :]
    tb = (np.abs(kb - qb) <= W)
    m = np.concatenate([ta, tb], axis=1).astype(np.float32)
    return np.concatenate([m, m], axis=1).astype(NPBF)


def _build_program(bias_flags):
    has_vqk_b, has_hq_b, has_hk_b, has_beff, has_b2 = bias_flags

    nc = bacc.Bacc("TRN2", target_bir_lowering=False, debug=False,
                   num_devices=NCORE, num_swdge_queues=4)

    xr_t = nc.dram_tensor("xr_t", [E, T], BF, kind="ExternalInput").ap()
    xc_t = nc.dram_tensor("xc_t", [E, T], BF, kind="ExternalInput").ap()
    w_vin = nc.dram_tensor("w_vin", [E, 3 * E], BF, kind="ExternalInput").ap()
    w_hq = nc.dram_tensor("w_hq", [E, E], BF, kind="ExternalInput").ap()
    w_hkv = nc.dram_tensor("w_hkv", [E, 2 * E], BF, kind="ExternalInput").ap()
    w_fh = nc.dram_tensor("w_fh", [E, E], BF, kind="ExternalInput").ap()
    w_fv = nc.dram_tensor("w_fv", [E, E], BF, kind="ExternalInput").ap()
    w_m2 = nc.dram_tensor("w_m2", [E, E], BF, kind="ExternalInput").ap()
    mask_d = nc.dram_tensor("mask", [128, 384], BF, kind="ExternalInput").ap()
    bias_d = nc.dram_tensor("biases", [128, 24], F32, kind="ExternalInput").ap()
    out_t = nc.dram_tensor("out_t", [E, T], BF, kind="ExternalOutput").ap()

    with tile.TileContext(nc) as tc, ExitStack() as ctx:
        pw = ctx.enter_context(tc.tile_pool(name="pw", bufs=1))
        psA = ctx.enter_context(tc.tile_pool(name="psA", bufs=3, space="PSUM"))
        psS = ctx.enter_context(tc.tile_pool(name="psS", bufs=1, space="PSUM"))
        psC = ctx.enter_context(tc.tile_pool(name="psC", bufs=2, space="PSUM"))
        px = ctx.enter_context(tc.tile_pool(name="px", bufs=3))
        pqk = ctx.enter_context(tc.tile_pool(name="pqk", bufs=32))
        pv = ctx.enter_context(tc.tile_pool(name="pv", bufs=8))
        pp = ctx.enter_context(tc.tile_pool(name="pp", bufs=20))
        pzr = ctx.enter_context(tc.tile_pool(name="pzr", bufs=12))
        pct = ctx.enter_context(tc.tile_pool(name="pct", bufs=3))
        phid = ctx.enter_context(tc.tile_pool(name="phid", bufs=8))
        pout = ctx.enter_context(tc.tile_pool(name="pout", bufs=8))

        def load_const(name, dram_ap, shape, dtype, eng=None):
            # weights split across the sync/scalar queues: the ~600ns issue
            # cost per DMA dominates startup if serialized on one queue
            t = pw.tile(shape, dtype, tag=name)
            (eng or nc.sync).dma_start(t[:], dram_ap)
            return t

        import os as _os
        NPAIR = int(_os.environ.get("BISPA_NPAIRS", RPC // 2))

        def load_x(g):
            # all xr tiles first: the group's first matmuls (h-branch Q)
            # need only xr; xc is first touched at j=4
            g0 = 2 * S * g
            xr2, xc2 = [], []
            for k in range(4):
                t = px.tile([128, 2 * S], BF, tag=f"xr{k}", name=f"xr{k}_{g}")
                nc.gpsimd.dma_start(t[:], xr_t[128 * k:128 * (k + 1), g0:g0 + 2 * S])
                xr2.append(t)
            for k in range(4):
                t = px.tile([128, 2 * S], BF, tag=f"xc{k}", name=f"xc{k}_{g}")
                nc.gpsimd.dma_start(t[:], xc_t[128 * k:128 * (k + 1), g0:g0 + 2 * S])
                xc2.append(t)
            return xr2, xc2

        xnext = load_x(0)

        # load order = consumption order: group-0 x tiles are queued first
        # (see load_x below); the first emitted matmuls are the h-branch QK
        # projections (whq, whkv), then v (wv), then attention constants,
        # then the MLP weights which are first needed one group later.
        whq = [load_const(f"whq{k}", w_hq[128 * k:128 * (k + 1), :], [128, E], BF)
               for k in range(4)]
        whkv = [load_const(f"whkv{k}", w_hkv[128 * k:128 * (k + 1), :], [128, 2 * E], BF)
                for k in range(4)]
        # wv split: QK columns land first (group-0 j-loop), V columns later
        wv = []
        for k in range(4):
            t = pw.tile([128, 3 * E], BF, tag=f"wv{k}")
            nc.sync.dma_start(t[:, 0:2 * E], w_vin[128 * k:128 * (k + 1), 0:2 * E])
            wv.append(t)
        for k in range(4):
            nc.sync.dma_start(wv[k][:, 2 * E:3 * E],
                              w_vin[128 * k:128 * (k + 1), 2 * E:3 * E])
        msk = load_const("msk", mask_d[:, :], [128, 384], BF)
        bia = load_const("bia", bias_d[:, :], [128, 24], F32)
        wfh = [load_const(f"wfh{k}", w_fh[128 * k:128 * (k + 1), :], [128, E], BF)
               for k in range(4)]
        wfv = [load_const(f"wfv{k}", w_fv[128 * k:128 * (k + 1), :], [128, E], BF)
               for k in range(4)]
        wm2 = [load_const(f"wm2{k}", w_m2[128 * k:128 * (k + 1), :], [128, E], BF)
               for k in range(4)]

        # bias cols: 0-7 v_in QK; 8-11 h Q; 12-15 h K; 16-19 b_eff; 20-23 b2

        prev_mlp = []   # deferred MLP closures from the previous group
        for g in range(NPAIR):
            g0 = 2 * S * g
            xr2, xc2 = xnext

            # ---------- QK projections, feature-major, N=384 ----------
            qk = {}
            for br in ("h", "v"):
                qk[br] = []
                for j in range(8):
                    ps = psA.tile([128, 384], F32, tag="proj",
                                  padded_shape=[128, 512])
                    for k in range(4):
                        if br == "v":
                            lhsT = wv[k][:, 128 * j:128 * (j + 1)]
                            rhs = xr2[k][:]
                        elif j < 4:
                            lhsT = whq[k][:, 128 * j:128 * (j + 1)]
                            rhs = xr2[k][:]
                        else:
                            lhsT = whkv[k][:, 128 * (j - 4):128 * (j - 3)]
                            rhs = xc2[k][:]
                        nc.tensor.matmul(ps[:], lhsT=lhsT, rhs=rhs,
                                         start=(k == 0), stop=(k == 3))
                    bcol = j if br == "v" else (8 + j)
                    has_b = ((has_vqk_b and br == "v")
                             or (has_hq_b and br == "h" and j < 4)
                             or (has_hk_b and br == "h" and j >= 4))
                    dst = pqk.tile([128, 384], BF, tag="qk")
                    if has_b:
                        nc.scalar.activation(dst[:], ps[:], AF.Identity,
                                             bias=bia[:, bcol:bcol + 1])
                    else:
                        # no bias: split evictions across engines by branch
                        if br == "h":
                            nc.vector.tensor_copy(dst[:], ps[:])
                        else:
                            nc.scalar.activation(dst[:], ps[:], AF.Identity)
                    qk[br].append(dst)

            if g + 1 < NPAIR:
                xnext = load_x(g + 1)

            # ct: [128 f, 8 f-blocks (0-3 h, 4-7 v), 2S tokens] single tile
            # so the feature-major turn is 2 XBAR DMA-transposes per strip.
            # Final group: one tile per strip instead — DMA-transpose write
            # tracking is tile-granular, and the strip-0 MLP half must not
            # wait on strip-1's transposes.
            if g + 1 < NPAIR:
                ct = pct.tile([128, 8, 2 * S], BF, tag="ct", name=f"ct_{g}")
                cts = [ct]
            else:
                cts = [pct.tile([128, 8, S], BF, tag=f"ctf{a}", name=f"ctf_{a}")
                       for a in range(2)]
                ct = None

            for a in range(2):
                s0 = S * a
                # ---- V projections for both branches first ----
                vab = {}
                for br in ("h", "v"):
                    xin = xr2 if br == "v" else xc2
                    vcols = slice(1024, 1536) if br == "v" else slice(512, 1024)
                    vw = wv if br == "v" else whkv
                    vps_a = psA.tile([128, 512], F32, tag="proj")
                    for k in range(4):
                        nc.tensor.matmul(vps_a[:], lhsT=xin[k][:, s0:s0 + 128],
                                         rhs=vw[k][:, vcols],
                                         start=(k == 0), stop=(k == 3))
                    vps_b = psA.tile([128, 512], F32, tag="proj")
                    for k in range(4):
                        nc.tensor.matmul(vps_b[:], lhsT=xin[k][:, s0 + 64:s0 + 192],
                                         rhs=vw[k][:, vcols],
                                         start=(k == 0), stop=(k == 3))
                    va = pv.tile([128, 8, 65], BF, tag="vp")
                    vb = pv.tile([128, 8, 65], BF, tag="vp")
                    nc.vector.tensor_copy(
                        va[:, :, 0:64],
                        vps_a[:].rearrange("p (h c) -> p h c", c=64))
                    nc.vector.tensor_copy(
                        vb[:, :, 0:64],
                        vps_b[:].rearrange("p (h c) -> p h c", c=64))
                    if g == 0:
                        # ones columns: group 0's 8 allocs touch all 8 pv
                        # buffers once; later strips only rewrite [:, :, 0:64]
                        nc.vector.memset(va[:, :, 64:65], 1.0)
                        nc.vector.memset(vb[:, :, 64:65], 1.0)
                    vab[br] = (va, vb)

                # ---- scores + exp + mask, pair-steps interleaved h/v ----
                # pm[br][p]: (128, 448) masked probs for heads 2p, 2p+1
                pm = {"h": [None] * 4, "v": [None] * 4}
                # attn@V bank plan: per br, 3 psC tiles:
                #   A: heads 0-2, B: heads 3-5, C: heads 6-7
                cxt = {"h": [None] * 3, "v": [None] * 3}
                zrs = {"h": [None] * 3, "v": [None] * 3}
                # normalized ctx, token-major: [96 q', 2 (h|v), 512 f] so one
                # XBAR transpose covers both branches
                ctxn1 = pp.tile([96, 2, 512], BF, tag="ctxn1", bufs=4)
                ctxn2 = pp.tile([96, 2, 512], BF, tag="ctxn2", bufs=4)
                BI = {"h": 0, "v": 1}

                def emit_pair(br, p, spt=None):
                    # both branches of a pair-step share one [128,1024] tile:
                    # bank A (cols 0:512) holds the h2=0 blocks of h and v,
                    # bank B the h2=1 blocks, keeping each PSUM bank row-
                    # group pure (quad row-tiles drain to distinct banks)
                    boff = 0 if br == "h" else 192
                    QT = qk[br][p][:, s0:s0 + S]
                    KT = qk[br][4 + p][:, s0:s0 + S]
                    mmsc = []
                    for h2 in range(2):
                        d0 = 64 * h2
                        c0 = 512 * h2 + boff
                        mmsc.append(nc.tensor.matmul(
                            spt[:, c0:c0 + 96],
                            lhsT=KT[d0:d0 + 64, 0:128],
                            rhs=QT[d0:d0 + 64, 0:96],
                            start=True, stop=True, skip_group_check=True))
                        mmsc.append(nc.tensor.matmul(
                            spt[:, c0 + 96:c0 + 192],
                            lhsT=KT[d0:d0 + 64, 64:192],
                            rhs=QT[d0:d0 + 64, 96:192],
                            start=True, stop=True, skip_group_check=True))
                    pb = pp.tile([128, 384], BF, tag="p")
                    sin = spt[:].rearrange("p (b c) -> p b c", c=512)[:, :, boff:boff + 192]
                    ex = nc.scalar.activation(
                        pb[:].rearrange("p (b c) -> p b c", c=192),
                        sin, AF.Exp, scale=0.125)
                    for m in mmsc:
                        add_dep_helper(ex.ins, m.ins, sync=True,
                                       reason="exp after score mms")
                    pmt = pp.tile([128, 384], BF, tag="p")
                    nc.vector.tensor_tensor(pmt[:], pb[:], msk[:], op=MUL)
                    pm[br][p] = pmt

                def bank_mms(br, b):
                    """attn@V bank b: heads hs = 3b..3b+2 (bank 2: h6,h7).
                    Per head 2 matmuls: q' [0,96) from TA keys [0,128) (va),
                    q' [96,192) from TB keys [64,192) (vb). Output rows 0:96,
                    head i at cols [130i, 130i+130) = [q1' 65 | q2' 65]."""
                    hs = [3 * b + i for i in range(3 if b < 2 else 2)]
                    va, vb = vab[br]
                    cp = psC.tile([128, 130 * len(hs)], F32, tag="cx",
                                  padded_shape=[128, 512], name=f"cp_{br}_{b}")
                    mms = []
                    n = 2 * len(hs)
                    for i, h in enumerate(hs):
                        pmt = pm[br][h // 2]
                        ta = 192 * (h % 2)
                        cb = 130 * i
                        mms.append(lambda i=i, h=h, pmt=pmt, ta=ta, cb=cb: nc.tensor.matmul(
                            cp[0:96, cb:cb + 65], lhsT=pmt[:, ta:ta + 96],
                            rhs=va[:, h:h + 1, :], start=(2 * i == 0),
                            stop=(2 * i == n - 1), skip_group_check=True))
                        mms.append(lambda i=i, h=h, pmt=pmt, ta=ta, cb=cb: nc.tensor.matmul(
                            cp[0:96, cb + 65:cb + 130],
                            lhsT=pmt[:, ta + 96:ta + 192],
                            rhs=vb[:, h:h + 1, :], start=(2 * i + 1 == 0),
                            stop=(2 * i + 1 == n - 1), skip_group_check=True))
                    return cp, mms, hs

                def emit_banks(b):
                    """Emit h and v banks with matmuls interleaved so the
                    per-matmul SBUF latency of one bank's chain overlaps the
                    other bank's execution (different PSUM banks)."""
                    cph, mmh, hs = bank_mms("h", b)
                    cpv, mmv, _ = bank_mms("v", b)
                    outh, outv = [], []
                    for fh, fv in zip(mmh, mmv):
                        outh.append(fh())
                        outv.append(fv())
                    _chain(outh)
                    _chain(outv)
                    cxt["h"][b] = (cph, outh[-1], hs)
                    cxt["v"][b] = (cpv, outv[-1], hs)

                def emit_norm(br, b):
                    # normalize: one reciprocal + two broadcast tensor_tensor
                    # ops per bank ([96, nh, 64] x stride-0 recip) instead of
                    # 2*nh small per-head muls split across DVE/ScalarE
                    cp, lastmm, hs = cxt[br][b]
                    nh = len(hs)
                    h0 = hs[0]
                    zr = pzr.tile([96, 2, nh, 1], F32, tag="zr")
                    # Z columns viewed (qblock, head): [96, 2, nh, 1]
                    cpz = cp[0:96, 0:130 * nh].rearrange(
                        "p (h q c) -> p q h c", q=2, c=65)
                    reads = [nc.vector.reciprocal(zr[:], cpz[:, :, :, 64:65])]
                    cph = cp[0:96, 0:130 * nh].rearrange("p (x c) -> p x c", c=130)
                    for qb, dst in ((0, ctxn1), (1, ctxn2)):
                        in0 = cph[:, :, 65 * qb:65 * qb + 64]
                        b0, b1 = bass.broadcast_tensor_aps(in0, zr[:, qb, :, :])
                        out = dst[0:96, BI[br], 64 * h0:64 * (h0 + nh)]
                        reads.append(nc.vector.tensor_tensor(
                            out.rearrange("p (x c) -> p x c", c=64),
                            b0, b1, op=MUL))
                    for r in reads:
                        add_dep_helper(r.ins, lastmm.ins, sync=True,
                                       reason="psum read after group close")

                # emission: pair-steps with attn@V banks interleaved; the
                # previous group's MLP units are sprinkled between steps so
                # the in-order PE always has a ready big matmul to chew on
                # skip early sites: strip 0 waits out the previous group's
                # in-flight ct transposes; strip 1 defers so the last units
                # cover the late-strip banks2/norm2 stalls (12 units, 16
                # sites -> pops at strip-0 sites 2-8, strip-1 sites 4-8)
                skip = [1 if a == 0 else 3]

                def mlp_step():
                    if skip[0] > 0:
                        skip[0] -= 1
                        return
                    if prev_mlp:
                        prev_mlp.pop(0)()

                spt0 = psS.tile([128, 1024], F32, tag="sc")
                for br in ("h", "v"):
                    emit_pair(br, 0, spt0)
                mlp_step()
                spt1 = psS.tile([128, 1024], F32, tag="sc")
                for br in ("h", "v"):
                    emit_pair(br, 1, spt1)
                mlp_step()
                emit_banks(0)             # heads 0-2 (needs pairs 0,1)
                mlp_step()
                spt2 = psS.tile([128, 1024], F32, tag="sc")
                for br in ("h", "v"):
                    emit_pair(br, 2, spt2)
                mlp_step()
                for br in ("h", "v"):
                    emit_norm(br, 0)
                mlp_step()
                emit_banks(1)             # heads 3-5 (needs pairs 1,2)
                mlp_step()
                spt3 = psS.tile([128, 1024], F32, tag="sc")
                for br in ("h", "v"):
                    emit_pair(br, 3, spt3)
                mlp_step()
                for br in ("h", "v"):
                    emit_norm(br, 1)
                emit_banks(2)             # heads 6,7
                mlp_step()
                for br in ("h", "v"):
                    emit_norm(br, 2)

                # ---- feature-major turn via XBAR DMA transpose: one call
                # per (strip, q-block) [96,1024] -> [128, 8, 96]; the
                # ~1.3us cost per call sits on the idle sync queue ----
                ctd = cts[0] if g + 1 < NPAIR else cts[a]
                c0 = s0 if g + 1 < NPAIR else 0
                nc.sync.dma_start_transpose(
                    ctd[:, :, c0:c0 + 96], ctxn1[:, :, :])
                nc.sync.dma_start_transpose(
                    ctd[:, :, c0 + 96:c0 + 192], ctxn2[:, :, :])

            # ---------- fused out-proj + MLP1 + MLP2 as deferred closures,
            # emitted interleaved into the NEXT group's attention ----------
            def build_mlp(ct=cts[0], g0=g0):
                units = []
                hid = []

                psj = {}

                def hid_unit_a(j):
                    ps = psA.tile([128, 384], F32, tag="proj",
                                  padded_shape=[128, 512], name=f"mlp1_{j}")
                    psj[j] = ps
                    for k in range(4):
                        nc.tensor.matmul(ps[:],
                                         lhsT=wfh[k][:, 128 * j:128 * (j + 1)],
                                         rhs=ct[:, k, :],
                                         start=(k == 0), stop=False)

                def hid_unit_b(j):
                    ps = psj[j]
                    for k in range(4):
                        nc.tensor.matmul(ps[:],
                                         lhsT=wfv[k][:, 128 * j:128 * (j + 1)],
                                         rhs=ct[:, 4 + k, :],
                                         start=False, stop=(k == 3))
                    dst = phid.tile([128, 384], BF, tag="hid", name=f"hid_{j}")
                    if has_beff:
                        nc.scalar.activation(dst[:], ps[:], AF.Relu,
                                             bias=bia[:, 16 + j:16 + j + 1])
                    else:
                        nc.scalar.activation(dst[:], ps[:], AF.Relu)
                    hid.append(dst)

                def out_unit(j):
                    ps = psA.tile([128, 384], F32, tag="proj",
                                  padded_shape=[128, 512], name=f"mlp2_{j}")
                    for k in range(4):
                        nc.tensor.matmul(ps[:],
                                         lhsT=wm2[k][:, 128 * j:128 * (j + 1)],
                                         rhs=hid[k][:],
                                         start=(k == 0), stop=(k == 3))
                    osb = pout.tile([128, 384], BF, tag="o", name=f"osb_{j}")
                    if has_b2:
                        nc.scalar.activation(osb[:], ps[:], AF.Identity,
                                             bias=bia[:, 20 + j:20 + j + 1])
                    else:
                        nc.scalar.activation(osb[:], ps[:], AF.Identity)
                    nc.sync.dma_start(
                        out_t[128 * j:128 * (j + 1), g0:g0 + 2 * S], osb[:])

                for j in range(4):
                    units.append(lambda j=j: hid_unit_a(j))
                    units.append(lambda j=j: hid_unit_b(j))
                for j in range(4):
                    units.append(lambda j=j: out_unit(j))
                return units

            def build_mlp_final(cts=cts, g0=g0):
                """Final group: token columns are independent through the MLP
                (contraction is over features), so strip-0's half runs as
                self-contained units during strip-1's attention; only the
                strip-1 half waits on the last ct transposes."""
                hid = {}

                def get_hid(j):
                    if j not in hid:
                        hid[j] = phid.tile([128, 384], BF, tag="hid",
                                           name=f"hidF_{j}")
                    return hid[j]

                def full_unit(j, a):
                    cta = cts[a]
                    c0 = S * a
                    ps = psA.tile([128, 384], F32, tag="proj",
                                  padded_shape=[128, 512], name=f"mlpF_{j}_{a}")
                    for k in range(4):
                        nc.tensor.matmul(ps[:, 0:S],
                                         lhsT=wfh[k][:, 128 * j:128 * (j + 1)],
                                         rhs=cta[:, k, :],
                                         start=(k == 0), stop=False)
                    for k in range(4):
                        nc.tensor.matmul(ps[:, 0:S],
                                         lhsT=wfv[k][:, 128 * j:128 * (j + 1)],
                                         rhs=cta[:, 4 + k, :],
                                         start=False, stop=(k == 3))
                    dst = get_hid(j)
                    if has_beff:
                        nc.scalar.activation(dst[:, c0:c0 + S], ps[:, 0:S],
                                             AF.Relu,
                                             bias=bia[:, 16 + j:16 + j + 1])
                    else:
                        nc.scalar.activation(dst[:, c0:c0 + S], ps[:, 0:S],
                                             AF.Relu)

                def out_unit(j):
                    ps = psA.tile([128, 384], F32, tag="proj",
                                  padded_shape=[128, 512], name=f"mlp2F_{j}")
                    for k in range(4):
                        nc.tensor.matmul(ps[:],
                                         lhsT=wm2[k][:, 128 * j:128 * (j + 1)],
                                         rhs=get_hid(k)[:],
                                         start=(k == 0), stop=(k == 3))
                    osb = pout.tile([128, 384], BF, tag="o", name=f"osbF_{j}")
                    if has_b2:
                        nc.scalar.activation(osb[:], ps[:], AF.Identity,
                                             bias=bia[:, 20 + j:20 + j + 1])
                    else:
                        nc.scalar.activation(osb[:], ps[:], AF.Identity)
                    nc.sync.dma_start(
                        out_t[128 * j:128 * (j + 1), g0:g0 + 2 * S], osb[:])

                u0 = [lambda j=j: full_unit(j, 0) for j in range(4)]
                u1 = [lambda j=j: full_unit(j, 1) for j in range(4)]
                u1 += [lambda j=j: out_unit(j) for j in range(4)]
                return u0, u1

            for f in prev_mlp:   # drain any leftovers (shouldn't happen)
                f()
            if g + 1 < NPAIR:
                prev_mlp = build_mlp()
            else:
                fin_u0, fin_u1 = build_mlp_final()
                prev_mlp = fin_u0 + fin_u1

        for f in prev_mlp:       # final group's MLP
            f()
    nc.finalize()
    return nc


_CACHE = {}


def _get_program(bias_flags):
    key = tuple(bias_flags)
    if key not in _CACHE:
        _CACHE[key] = _build_program(key)
    return _CACHE[key]


def _col(b):
    return np.ascontiguousarray(b.reshape(-1, 128).T.astype(np.float32))


def kernel(hidden_states, h_in_w, h_in_b, h_out_w, h_out_b,
           v_in_w, v_in_b, v_out_w, v_out_b,
           mlp_w1, mlp_b1, mlp_w2, mlp_b2):
    x = np.asarray(hidden_states, dtype=np.float32)
    h_in_w = np.asarray(h_in_w, np.float32)
    h_in_b = np.asarray(h_in_b, np.float32)
    h_out_w = np.asarray(h_out_w, np.float32)
    h_out_b = np.asarray(h_out_b, np.float32)
    v_in_w = np.asarray(v_in_w, np.float32)
    v_in_b = np.asarray(v_in_b, np.float32)
    v_out_w = np.asarray(v_out_w, np.float32)
    v_out_b = np.asarray(v_out_b, np.float32)
    mlp_w1 = np.asarray(mlp_w1, np.float32)
    mlp_b1 = np.asarray(mlp_b1, np.float32)
    mlp_w2 = np.asarray(mlp_w2, np.float32)
    mlp_b2 = np.asarray(mlp_b2, np.float32)

    # V biases shift ctx by a constant (softmax weights sum to 1): fold through
    # out-proj; then fold out-proj entirely into MLP1 (relu is the only
    # nonlinearity after it): hid = relu(h_ctx@Wfh^T + v_ctx@Wfv^T + b_eff).
    h_out_eff = h_out_b + h_out_w @ h_in_b[2 * E:3 * E]
    v_out_eff = v_out_b + v_out_w @ v_in_b[2 * E:3 * E]
    W1h = mlp_w1[:, 0:E]
    W1v = mlp_w1[:, E:2 * E]
    Wfh = W1h @ h_out_w            # (E, E)
    Wfv = W1v @ v_out_w
    b_eff = mlp_b1 + W1h @ h_out_eff + W1v @ v_out_eff

    bias_flags = (
        bool(np.any(v_in_b[0:2 * E])), bool(np.any(h_in_b[0:E])),
        bool(np.any(h_in_b[E:2 * E])), bool(np.any(b_eff)),
        bool(np.any(mlp_b2)),
    )
    nc = _get_program(bias_flags)

    biases = np.zeros((128, 24), np.float32)
    biases[:, 0:8] = _col(v_in_b[0:2 * E])
    biases[:, 8:16] = _col(h_in_b[0:2 * E])
    biases[:, 16:20] = _col(b_eff)
    biases[:, 20:24] = _col(mlp_b2)

    shared = {
        "w_vin": np.ascontiguousarray(v_in_w.T).astype(NPBF),
        "w_hq": np.ascontiguousarray(h_in_w[0:E].T).astype(NPBF),
        "w_hkv": np.ascontiguousarray(h_in_w[E:3 * E].T).astype(NPBF),
        "w_fh": np.ascontiguousarray(Wfh.T).astype(NPBF),
        "w_fv": np.ascontiguousarray(Wfv.T).astype(NPBF),
        "w_m2": np.ascontiguousarray(mlp_w2.T).astype(NPBF),
        "mask": _band_masks(),
        "biases": biases,
    }

    in_maps = []
    for c in range(NCORE):
        rows = x[RPC * c:RPC * (c + 1)]
        cols = x[:, RPC * c:RPC * (c + 1)].transpose(1, 0, 2)
        m = dict(shared)
        m["xr_t"] = np.ascontiguousarray(rows.reshape(T, E).T).astype(NPBF)
        m["xc_t"] = np.ascontiguousarray(cols.reshape(T, E).T).astype(NPBF)
        in_maps.append(m)

    global _LAST_IN_MAPS
    _LAST_IN_MAPS = in_maps
    res = run_bass_kernel_spmd(nc, in_maps, core_ids=list(range(NCORE)))

    out = np.empty((S, S, E), np.float32)
    for c in range(NCORE):
        out[RPC * c:RPC * (c + 1)] = res.results[c]["out_t"].astype(np.float32).T.reshape(RPC, S, E)
    return out



# revision 52
# speedup vs baseline: 1.0235x; 1.0235x over previous
"""BiSPA (bidirectional sparse windowed attention + MLP) Trainium2 kernel.

Full inputs in, full outputs out; core c owns output rows [24c, 24c+24).
Optimized v9 (~458-465us at full clock vs 511us v8 / 1065us baseline;
P0 throttle phases add up to ~10%). Key ideas on top of v8:
- feature-major turn (ctx token-major -> MLP feature-major) moved off the
  PE: XBAR DMA-transposes on the sync queue ([96,1024] -> [128,8,96] into
  a single per-group ct tile), replacing 32 PE transposes + 8 DVE copies
  per group. PSUM freed by pcx -> psS double-buffered.
- softmax normalization as stride-0-broadcast DVE tensor_tensor
  ([96, nh, 64] ctx times recip[96, nh, 1]): 2 ops per attn@V bank
  instead of 2*nh per-head muls split across DVE/ScalarE (ScalarE busy
  68% -> 45%).
- score pairs packed 2-per-[128,1024] tile with h2=0/h2=1 blocks in
  separate banks: quad row-tiles drain to distinct PSUM banks, so each
  bank must stay row-group pure (mixing row groups in a bank hangs the
  NEFF). Frees 2 banks -> psA triple-buffered.
- final group's MLP split per strip (token columns are independent):
  strip-0 half runs while strip-1 attention/transposes finish.
- startup: weights on the sync queue in consumption order (whq, whkv,
  wv-QK | wv-V split), xr before xc; first matmul at ~10us.
- attn@V ones-columns written once (group 0 touches all 8 pv buffers).
- output DMA'd as bf16 (half the bytes), host converts to fp32.
Known limits: scores/attn@V are LDWEIGHTS-port-bound (~80-90ns/matmul vs
40ns stream); fp8 fails the 2e-2 gate (V-proj/MLP2 ~3.4-4.3e-2 alone);
ctx^T attn@V dead-ends on partition-broadcast for Z (DVE lanes are
partition-locked, gpsimd ops cost ~14.5us each).
"""

import numpy as np
from contextlib import ExitStack

import concourse.bass as bass
import concourse.mybir as mybir
import concourse.tile as tile
from concourse import bacc
from concourse.bass_utils import run_bass_kernel_spmd
from concourse.tile import add_dep_helper


def _chain(insts):
    for a, b in zip(insts, insts[1:]):
        add_dep_helper(b.ins, a.ins, sync=False, reason="psum-bank group order")

BF = mybir.dt.bfloat16
F32 = mybir.dt.float32
AF = mybir.ActivationFunctionType
MUL = mybir.AluOpType.mult
NPBF = mybir.dt.np(BF)

E = 512
H = 8
D = 64
W = 32
S = 192
NCORE = 8
RPC = 24
T = RPC * S


def _band_masks():
    """Score mask, bf16 (128, 384): [TA 96 | TB 96] x 2 heads.

    q-blocks split at 96 so neither straddles a key block:
      TA: rows k in [0,128), cols q in [0,96):    valid = |k-q| <= W
      TB: rows k = 64+r in [64,192), cols q = 96+c in [96,192):
          valid = |k-q| <= W
    """
    k = np.arange(128)[:, None]
    qa = np.arange(96)[None, :]
    ta = (np.abs(k - qa) <= W)
    kb = 64 + np.arange(128)[:, None]
    qb = 96 + np.arange(96)[None, :]
    tb = (np.abs(kb - qb) <= W)
    m = np.concatenate([ta, tb], axis=1).astype(np.float32)
    return np.concatenate([m, m], axis=1).astype(NPBF)


def _build_program(bias_flags):
    has_vqk_b, has_hq_b, has_hk_b, has_beff, has_b2 = bias_flags

    nc = bacc.Bacc("TRN2", target_bir_lowering=False, debug=False,
                   num_devices=NCORE, num_swdge_queues=4)

    xr_t = nc.dram_tensor("xr_t", [E, T], BF, kind="ExternalInput").ap()
    xc_t = nc.dram_tensor("xc_t", [E, T], BF, kind="ExternalInput").ap()
    w_vin = nc.dram_tensor("w_vin", [E, 3 * E], BF, kind="ExternalInput").ap()
    w_hq = nc.dram_tensor("w_hq", [E, E], BF, kind="ExternalInput").ap()
    w_hkv = nc.dram_tensor("w_hkv", [E, 2 * E], BF, kind="ExternalInput").ap()
    w_fh = nc.dram_tensor("w_fh", [E, E], BF, kind="ExternalInput").ap()
    w_fv = nc.dram_tensor("w_fv", [E, E], BF, kind="ExternalInput").ap()
    w_m2 = nc.dram_tensor("w_m2", [E, E], BF, kind="ExternalInput").ap()
    mask_d = nc.dram_tensor("mask", [128, 384], BF, kind="ExternalInput").ap()
    bias_d = nc.dram_tensor("biases", [128, 24], F32, kind="ExternalInput").ap()
    out_t = nc.dram_tensor("out_t", [E, T], BF, kind="ExternalOutput").ap()

    with tile.TileContext(nc) as tc, ExitStack() as ctx:
        pw = ctx.enter_context(tc.tile_pool(name="pw", bufs=1))
        psA = ctx.enter_context(tc.tile_pool(name="psA", bufs=4, space="PSUM"))
        psS = ctx.enter_context(tc.tile_pool(name="psS", bufs=1, space="PSUM"))
        psC = ctx.enter_context(tc.tile_pool(name="psC", bufs=2, space="PSUM"))
        px = ctx.enter_context(tc.tile_pool(name="px", bufs=3))
        pqk = ctx.enter_context(tc.tile_pool(name="pqk", bufs=32))
        pv = ctx.enter_context(tc.tile_pool(name="pv", bufs=8))
        pp = ctx.enter_context(tc.tile_pool(name="pp", bufs=20))
        pzr = ctx.enter_context(tc.tile_pool(name="pzr", bufs=12))
        pct = ctx.enter_context(tc.tile_pool(name="pct", bufs=3))
        phid = ctx.enter_context(tc.tile_pool(name="phid", bufs=8))
        pout = ctx.enter_context(tc.tile_pool(name="pout", bufs=8))

        def load_const(name, dram_ap, shape, dtype, eng=None):
            # weights split across the sync/scalar queues: the ~600ns issue
            # cost per DMA dominates startup if serialized on one queue
            t = pw.tile(shape, dtype, tag=name)
            (eng or nc.sync).dma_start(t[:], dram_ap)
            return t

        import os as _os
        NPAIR = int(_os.environ.get("BISPA_NPAIRS", RPC // 2))

        def load_x(g):
            # all xr tiles first: the group's first matmuls (h-branch Q)
            # need only xr; xc is first touched at j=4
            g0 = 2 * S * g
            xr2, xc2 = [], []
            for k in range(4):
                t = px.tile([128, 2 * S], BF, tag=f"xr{k}", name=f"xr{k}_{g}")
                nc.gpsimd.dma_start(t[:], xr_t[128 * k:128 * (k + 1), g0:g0 + 2 * S])
                xr2.append(t)
            for k in range(4):
                t = px.tile([128, 2 * S], BF, tag=f"xc{k}", name=f"xc{k}_{g}")
                nc.gpsimd.dma_start(t[:], xc_t[128 * k:128 * (k + 1), g0:g0 + 2 * S])
                xc2.append(t)
            return xr2, xc2

        xnext = load_x(0)

        # load order = consumption order: group-0 x tiles are queued first
        # (see load_x below); the first emitted matmuls are the h-branch QK
        # projections (whq, whkv), then v (wv), then attention constants,
        # then the MLP weights which are first needed one group later.
        whq = [load_const(f"whq{k}", w_hq[128 * k:128 * (k + 1), :], [128, E], BF)
               for k in range(4)]
        whkv = [load_const(f"whkv{k}", w_hkv[128 * k:128 * (k + 1), :], [128, 2 * E], BF)
                for k in range(4)]
        # wv split: QK columns land first (group-0 j-loop), V columns later
        wv = []
        for k in range(4):
            t = pw.tile([128, 3 * E], BF, tag=f"wv{k}")
            nc.sync.dma_start(t[:, 0:2 * E], w_vin[128 * k:128 * (k + 1), 0:2 * E])
            wv.append(t)
        for k in range(4):
            nc.sync.dma_start(wv[k][:, 2 * E:3 * E],
                              w_vin[128 * k:128 * (k + 1), 2 * E:3 * E])
        msk = load_const("msk", mask_d[:, :], [128, 384], BF)
        bia = load_const("bia", bias_d[:, :], [128, 24], F32)
        wfh = [load_const(f"wfh{k}", w_fh[128 * k:128 * (k + 1), :], [128, E], BF)
               for k in range(4)]
        wfv = [load_const(f"wfv{k}", w_fv[128 * k:128 * (k + 1), :], [128, E], BF)
               for k in range(4)]
        wm2 = [load_const(f"wm2{k}", w_m2[128 * k:128 * (k + 1), :], [128, E], BF)
               for k in range(4)]

        # bias cols: 0-7 v_in QK; 8-11 h Q; 12-15 h K; 16-19 b_eff; 20-23 b2

        prev_mlp = []   # deferred MLP closures from the previous group
        for g in range(NPAIR):
            g0 = 2 * S * g
            xr2, xc2 = xnext

            # ---------- QK projections, feature-major, N=384 ----------
            qk = {}
            for br in ("h", "v"):
                qk[br] = []
                for j in range(8):
                    ps = psA.tile([128, 384], F32, tag="proj",
                                  padded_shape=[128, 512])
                    for k in range(4):
                        if br == "v":
                            lhsT = wv[k][:, 128 * j:128 * (j + 1)]
                            rhs = xr2[k][:]
                        elif j < 4:
                            lhsT = whq[k][:, 128 * j:128 * (j + 1)]
                            rhs = xr2[k][:]
                        else:
                            lhsT = whkv[k][:, 128 * (j - 4):128 * (j - 3)]
                            rhs = xc2[k][:]
                        nc.tensor.matmul(ps[:], lhsT=lhsT, rhs=rhs,
                                         start=(k == 0), stop=(k == 3))
                    bcol = j if br == "v" else (8 + j)
                    has_b = ((has_vqk_b and br == "v")
                             or (has_hq_b and br == "h" and j < 4)
                             or (has_hk_b and br == "h" and j >= 4))
                    dst = pqk.tile([128, 384], BF, tag="qk")
                    if has_b:
                        nc.scalar.activation(dst[:], ps[:], AF.Identity,
                                             bias=bia[:, bcol:bcol + 1])
                    else:
                        # no bias: split evictions across engines by branch
                        if br == "h":
                            nc.vector.tensor_copy(dst[:], ps[:])
                        else:
                            nc.scalar.activation(dst[:], ps[:], AF.Identity)
                    qk[br].append(dst)

            if g + 1 < NPAIR:
                xnext = load_x(g + 1)

            # ct: [128 f, 8 f-blocks (0-3 h, 4-7 v), 2S tokens] single tile
            # so the feature-major turn is 2 XBAR DMA-transposes per strip.
            # Final group: one tile per strip instead — DMA-transpose write
            # tracking is tile-granular, and the strip-0 MLP half must not
            # wait on strip-1's transposes.
            if g + 1 < NPAIR:
                ct = pct.tile([128, 8, 2 * S], BF, tag="ct", name=f"ct_{g}")
                cts = [ct]
            else:
                cts = [pct.tile([128, 8, S], BF, tag=f"ctf{a}", name=f"ctf_{a}")
                       for a in range(2)]
                ct = None

            for a in range(2):
                s0 = S * a
                # ---- V projections for both branches first ----
                vab = {}
                for br in ("h", "v"):
                    xin = xr2 if br == "v" else xc2
                    vcols = slice(1024, 1536) if br == "v" else slice(512, 1024)
                    vw = wv if br == "v" else whkv
                    vps_a = psA.tile([128, 512], F32, tag="proj")
                    for k in range(4):
                        nc.tensor.matmul(vps_a[:], lhsT=xin[k][:, s0:s0 + 128],
                                         rhs=vw[k][:, vcols],
                                         start=(k == 0), stop=(k == 3))
                    vps_b = psA.tile([128, 512], F32, tag="proj")
                    for k in range(4):
                        nc.tensor.matmul(vps_b[:], lhsT=xin[k][:, s0 + 64:s0 + 192],
                                         rhs=vw[k][:, vcols],
                                         start=(k == 0), stop=(k == 3))
                    va = pv.tile([128, 8, 65], BF, tag="vp")
                    vb = pv.tile([128, 8, 65], BF, tag="vp")
                    nc.vector.tensor_copy(
                        va[:, :, 0:64],
                        vps_a[:].rearrange("p (h c) -> p h c", c=64))
                    nc.vector.tensor_copy(
                        vb[:, :, 0:64],
                        vps_b[:].rearrange("p (h c) -> p h c", c=64))
                    if g == 0:
                        # ones columns: group 0's 8 allocs touch all 8 pv
                        # buffers once; later strips only rewrite [:, :, 0:64]
                        nc.vector.memset(va[:, :, 64:65], 1.0)
                        nc.vector.memset(vb[:, :, 64:65], 1.0)
                    vab[br] = (va, vb)

                # ---- scores + exp + mask, pair-steps interleaved h/v ----
                # pm[br][p]: (128, 448) masked probs for heads 2p, 2p+1
                pm = {"h": [None] * 4, "v": [None] * 4}
                # attn@V bank plan: per br, 3 psC tiles:
                #   A: heads 0-2, B: heads 3-5, C: heads 6-7
                cxt = {"h": [None] * 3, "v": [None] * 3}
                zrs = {"h": [None] * 3, "v": [None] * 3}
                # normalized ctx, token-major: [96 q', 2 (h|v), 512 f] so one
                # XBAR transpose covers both branches
                ctxn1 = pp.tile([96, 2, 512], BF, tag="ctxn1", bufs=4)
                ctxn2 = pp.tile([96, 2, 512], BF, tag="ctxn2", bufs=4)
                BI = {"h": 0, "v": 1}

                def emit_pair(br, p, spt=None):
                    # both branches of a pair-step share one [128,1024] tile:
                    # bank A (cols 0:512) holds the h2=0 blocks of h and v,
                    # bank B the h2=1 blocks, keeping each PSUM bank row-
                    # group pure (quad row-tiles drain to distinct banks)
                    boff = 0 if br == "h" else 192
                    QT = qk[br][p][:, s0:s0 + S]
                    KT = qk[br][4 + p][:, s0:s0 + S]
                    mmsc = []
                    for h2 in range(2):
                        d0 = 64 * h2
                        c0 = 512 * h2 + boff
                        mmsc.append(nc.tensor.matmul(
                            spt[:, c0:c0 + 96],
                            lhsT=KT[d0:d0 + 64, 0:128],
                            rhs=QT[d0:d0 + 64, 0:96],
                            start=True, stop=True, skip_group_check=True))
                        mmsc.append(nc.tensor.matmul(
                            spt[:, c0 + 96:c0 + 192],
                            lhsT=KT[d0:d0 + 64, 64:192],
                            rhs=QT[d0:d0 + 64, 96:192],
                            start=True, stop=True, skip_group_check=True))
                    pb = pp.tile([128, 384], BF, tag="p")
                    sin = spt[:].rearrange("p (b c) -> p b c", c=512)[:, :, boff:boff + 192]
                    ex = nc.scalar.activation(
                        pb[:].rearrange("p (b c) -> p b c", c=192),
                        sin, AF.Exp, scale=0.125)
                    for m in mmsc:
                        add_dep_helper(ex.ins, m.ins, sync=True,
                                       reason="exp after score mms")
                    pmt = pp.tile([128, 384], BF, tag="p")
                    nc.vector.tensor_tensor(pmt[:], pb[:], msk[:], op=MUL)
                    pm[br][p] = pmt

                def bank_mms(br, b):
                    """attn@V bank b: heads hs = 3b..3b+2 (bank 2: h6,h7).
                    Per head 2 matmuls: q' [0,96) from TA keys [0,128) (va),
                    q' [96,192) from TB keys [64,192) (vb). Output rows 0:96,
                    head i at cols [130i, 130i+130) = [q1' 65 | q2' 65]."""
                    hs = [3 * b + i for i in range(3 if b < 2 else 2)]
                    va, vb = vab[br]
                    cp = psC.tile([128, 130 * len(hs)], F32, tag="cx",
                                  padded_shape=[128, 512], name=f"cp_{br}_{b}")
                    mms = []
                    n = 2 * len(hs)
                    for i, h in enumerate(hs):
                        pmt = pm[br][h // 2]
                        ta = 192 * (h % 2)
                        cb = 130 * i
                        mms.append(lambda i=i, h=h, pmt=pmt, ta=ta, cb=cb: nc.tensor.matmul(
                            cp[0:96, cb:cb + 65], lhsT=pmt[:, ta:ta + 96],
                            rhs=va[:, h:h + 1, :], start=(2 * i == 0),
                            stop=(2 * i == n - 1), skip_group_check=True))
                        mms.append(lambda i=i, h=h, pmt=pmt, ta=ta, cb=cb: nc.tensor.matmul(
                            cp[0:96, cb + 65:cb + 130],
                            lhsT=pmt[:, ta + 96:ta + 192],
                            rhs=vb[:, h:h + 1, :], start=(2 * i + 1 == 0),
                            stop=(2 * i + 1 == n - 1), skip_group_check=True))
                    return cp, mms, hs

                def emit_banks(b):
                    """Emit h and v banks with matmuls interleaved so the
                    per-matmul SBUF latency of one bank's chain overlaps the
                    other bank's execution (different PSUM banks)."""
                    cph, mmh, hs = bank_mms("h", b)
                    cpv, mmv, _ = bank_mms("v", b)
                    outh, outv = [], []
                    for fh, fv in zip(mmh, mmv):
                        outh.append(fh())
                        outv.append(fv())
                    _chain(outh)
                    _chain(outv)
                    cxt["h"][b] = (cph, outh[-1], hs)
                    cxt["v"][b] = (cpv, outv[-1], hs)

                def emit_norm(br, b):
                    # normalize: one reciprocal + two broadcast tensor_tensor
                    # ops per bank ([96, nh, 64] x stride-0 recip) instead of
                    # 2*nh small per-head muls split across DVE/ScalarE
                    cp, lastmm, hs = cxt[br][b]
                    nh = len(hs)
                    h0 = hs[0]
                    zr = pzr.tile([96, 2, nh, 1], F32, tag="zr")
                    # Z columns viewed (qblock, head): [96, 2, nh, 1]
                    cpz = cp[0:96, 0:130 * nh].rearrange(
                        "p (h q c) -> p q h c", q=2, c=65)
                    reads = [nc.vector.reciprocal(zr[:], cpz[:, :, :, 64:65])]
                    cph = cp[0:96, 0:130 * nh].rearrange("p (x c) -> p x c", c=130)
                    for qb, dst in ((0, ctxn1), (1, ctxn2)):
                        in0 = cph[:, :, 65 * qb:65 * qb + 64]
                        b0, b1 = bass.broadcast_tensor_aps(in0, zr[:, qb, :, :])
                        out = dst[0:96, BI[br], 64 * h0:64 * (h0 + nh)]
                        reads.append(nc.vector.tensor_tensor(
                            out.rearrange("p (x c) -> p x c", c=64),
                            b0, b1, op=MUL))
                    for r in reads:
                        add_dep_helper(r.ins, lastmm.ins, sync=True,
                                       reason="psum read after group close")

                # emission: pair-steps with attn@V banks interleaved; the
                # previous group's MLP units are sprinkled between steps so
                # the in-order PE always has a ready big matmul to chew on
                # skip early sites: strip 0 waits out the previous group's
                # in-flight ct transposes; strip 1 defers so the last units
                # cover the late-strip banks2/norm2 stalls (12 units, 16
                # sites -> pops at strip-0 sites 2-8, strip-1 sites 4-8)
                skip = [1 if a == 0 else 3]

                def mlp_step():
                    if skip[0] > 0:
                        skip[0] -= 1
                        return
                    if prev_mlp:
                        prev_mlp.pop(0)()

                spt0 = psS.tile([128, 1024], F32, tag="sc")
                for br in ("h", "v"):
                    emit_pair(br, 0, spt0)
                mlp_step()
                spt1 = psS.tile([128, 1024], F32, tag="sc")
                for br in ("h", "v"):
                    emit_pair(br, 1, spt1)
                mlp_step()
                emit_banks(0)             # heads 0-2 (needs pairs 0,1)
                mlp_step()
                spt2 = psS.tile([128, 1024], F32, tag="sc")
                for br in ("h", "v"):
                    emit_pair(br, 2, spt2)
                mlp_step()
                for br in ("h", "v"):
                    emit_norm(br, 0)
                mlp_step()
                emit_banks(1)             # heads 3-5 (needs pairs 1,2)
                mlp_step()
                spt3 = psS.tile([128, 1024], F32, tag="sc")
                for br in ("h", "v"):
                    emit_pair(br, 3, spt3)
                mlp_step()
                for br in ("h", "v"):
                    emit_norm(br, 1)
                emit_banks(2)             # heads 6,7
                mlp_step()
                for br in ("h", "v"):
                    emit_norm(br, 2)

                # ---- feature-major turn via XBAR DMA transpose: one call
                # per (strip, q-block) [96,1024] -> [128, 8, 96]; the
                # ~1.3us cost per call sits on the idle sync queue ----
                ctd = cts[0] if g + 1 < NPAIR else cts[a]
                c0 = s0 if g + 1 < NPAIR else 0
                nc.sync.dma_start_transpose(
                    ctd[:, :, c0:c0 + 96], ctxn1[:, :, :])
                nc.sync.dma_start_transpose(
                    ctd[:, :, c0 + 96:c0 + 192], ctxn2[:, :, :])

            # ---------- fused out-proj + MLP1 + MLP2 as deferred closures,
            # emitted interleaved into the NEXT group's attention ----------
            def build_mlp(ct=cts[0], g0=g0):
                units = []
                hid = []

                psj = {}

                def hid_unit_a(j):
                    ps = psA.tile([128, 384], F32, tag="proj",
                                  padded_shape=[128, 512], name=f"mlp1_{j}")
                    psj[j] = ps
                    for k in range(4):
                        nc.tensor.matmul(ps[:],
                                         lhsT=wfh[k][:, 128 * j:128 * (j + 1)],
                                         rhs=ct[:, k, :],
                                         start=(k == 0), stop=False)

                def hid_unit_b(j):
                    ps = psj[j]
                    for k in range(4):
                        nc.tensor.matmul(ps[:],
                                         lhsT=wfv[k][:, 128 * j:128 * (j + 1)],
                                         rhs=ct[:, 4 + k, :],
                                         start=False, stop=(k == 3))
                    dst = phid.tile([128, 384], BF, tag="hid", name=f"hid_{j}")
                    if has_beff:
                        nc.scalar.activation(dst[:], ps[:], AF.Relu,
                                             bias=bia[:, 16 + j:16 + j + 1])
                    else:
                        nc.scalar.activation(dst[:], ps[:], AF.Relu)
                    hid.append(dst)

                def out_unit(j):
                    ps = psA.tile([128, 384], F32, tag="proj",
                                  padded_shape=[128, 512], name=f"mlp2_{j}")
                    for k in range(4):
                        nc.tensor.matmul(ps[:],
                                         lhsT=wm2[k][:, 128 * j:128 * (j + 1)],
                                         rhs=hid[k][:],
                                         start=(k == 0), stop=(k == 3))
                    osb = pout.tile([128, 384], BF, tag="o", name=f"osb_{j}")
                    if has_b2:
                        nc.scalar.activation(osb[:], ps[:], AF.Identity,
                                             bias=bia[:, 20 + j:20 + j + 1])
                    else:
                        nc.scalar.activation(osb[:], ps[:], AF.Identity)
                    nc.sync.dma_start(
                        out_t[128 * j:128 * (j + 1), g0:g0 + 2 * S], osb[:])

                for j in range(4):
                    units.append(lambda j=j: hid_unit_a(j))
                    units.append(lambda j=j: hid_unit_b(j))
                for j in range(4):
                    units.append(lambda j=j: out_unit(j))
                return units

            def build_mlp_final(cts=cts, g0=g0):
                """Final group: token columns are independent through the MLP
                (contraction is over features), so strip-0's half runs as
                self-contained units during strip-1's attention; only the
                strip-1 half waits on the last ct transposes."""
                hid = {}

                def get_hid(j):
                    if j not in hid:
                        hid[j] = phid.tile([128, 384], BF, tag="hid",
                                           name=f"hidF_{j}")
                    return hid[j]

                def full_unit(j, a):
                    cta = cts[a]
                    c0 = S * a
                    ps = psA.tile([128, 384], F32, tag="proj",
                                  padded_shape=[128, 512], name=f"mlpF_{j}_{a}")
                    for k in range(4):
                        nc.tensor.matmul(ps[:, 0:S],
                                         lhsT=wfh[k][:, 128 * j:128 * (j + 1)],
                                         rhs=cta[:, k, :],
                                         start=(k == 0), stop=False)
                    for k in range(4):
                        nc.tensor.matmul(ps[:, 0:S],
                                         lhsT=wfv[k][:, 128 * j:128 * (j + 1)],
                                         rhs=cta[:, 4 + k, :],
                                         start=False, stop=(k == 3))
                    dst = get_hid(j)
                    if has_beff:
                        nc.scalar.activation(dst[:, c0:c0 + S], ps[:, 0:S],
                                             AF.Relu,
                                             bias=bia[:, 16 + j:16 + j + 1])
                    else:
                        nc.scalar.activation(dst[:, c0:c0 + S], ps[:, 0:S],
                                             AF.Relu)

                def out_unit(j):
                    ps = psA.tile([128, 384], F32, tag="proj",
                                  padded_shape=[128, 512], name=f"mlp2F_{j}")
                    for k in range(4):
                        nc.tensor.matmul(ps[:],
                                         lhsT=wm2[k][:, 128 * j:128 * (j + 1)],
                                         rhs=get_hid(k)[:],
                                         start=(k == 0), stop=(k == 3))
                    osb = pout.tile([128, 384], BF, tag="o", name=f"osbF_{j}")
                    if has_b2:
                        nc.scalar.activation(osb[:], ps[:], AF.Identity,
                                             bias=bia[:, 20 + j:20 + j + 1])
                    else:
                        nc.scalar.activation(osb[:], ps[:], AF.Identity)
                    nc.sync.dma_start(
                        out_t[128 * j:128 * (j + 1), g0:g0 + 2 * S], osb[:])

                u0 = [lambda j=j: full_unit(j, 0) for j in range(4)]
                u1 = [lambda j=j: full_unit(j, 1) for j in range(4)]
                u1 += [lambda j=j: out_unit(j) for j in range(4)]
                return u0, u1

            for f in prev_mlp:   # drain any leftovers (shouldn't happen)
                f()
            if g + 1 < NPAIR:
                prev_mlp = build_mlp()
            else:
                fin_u0, fin_u1 = build_mlp_final()
                prev_mlp = fin_u0 + fin_u1

        for f in prev_mlp:       # final group's MLP
            f()
    nc.finalize()
    return nc


_CACHE = {}


def _get_program(bias_flags):
    key = tuple(bias_flags)
    if key not in _CACHE:
        _CACHE[key] = _build_program(key)
    return _CACHE[key]


def _col(b):
    return np.ascontiguousarray(b.reshape(-1, 128).T.astype(np.float32))


def kernel(hidden_states, h_in_w, h_in_b, h_out_w, h_out_b,
           v_in_w, v_in_b, v_out_w, v_out_b,
           mlp_w1, mlp_b1, mlp_w2, mlp_b2):
    x = np.asarray(hidden_states, dtype=np.float32)
    h_in_w = np.asarray(h_in_w, np.float32)
    h_in_b = np.asarray(h_in_b, np.float32)
    h_out_w = np.asarray(h_out_w, np.float32)
    h_out_b = np.asarray(h_out_b, np.float32)
    v_in_w = np.asarray(v_in_w, np.float32)
    v_in_b = np.asarray(v_in_b, np.float32)
    v_out_w = np.asarray(v_out_w, np.float32)
    v_out_b = np.asarray(v_out_b, np.float32)
    mlp_w1 = np.asarray(mlp_w1, np.float32)
    mlp_b1 = np.asarray(mlp_b1, np.float32)
    mlp_w2 = np.asarray(mlp_w2, np.float32)
    mlp_b2 = np.asarray(mlp_b2, np.float32)

    # V biases shift ctx by a constant (softmax weights sum to 1): fold through
    # out-proj; then fold out-proj entirely into MLP1 (relu is the only
    # nonlinearity after it): hid = relu(h_ctx@Wfh^T + v_ctx@Wfv^T + b_eff).
    h_out_eff = h_out_b + h_out_w @ h_in_b[2 * E:3 * E]
    v_out_eff = v_out_b + v_out_w @ v_in_b[2 * E:3 * E]
    W1h = mlp_w1[:, 0:E]
    W1v = mlp_w1[:, E:2 * E]
    Wfh = W1h @ h_out_w            # (E, E)
    Wfv = W1v @ v_out_w
    b_eff = mlp_b1 + W1h @ h_out_eff + W1v @ v_out_eff

    bias_flags = (
        bool(np.any(v_in_b[0:2 * E])), bool(np.any(h_in_b[0:E])),
        bool(np.any(h_in_b[E:2 * E])), bool(np.any(b_eff)),
        bool(np.any(mlp_b2)),
    )
    nc = _get_program(bias_flags)

    biases = np.zeros((128, 24), np.float32)
    biases[:, 0:8] = _col(v_in_b[0:2 * E])
    biases[:, 8:16] = _col(h_in_b[0:2 * E])
    biases[:, 16:20] = _col(b_eff)
    biases[:, 20:24] = _col(mlp_b2)

    shared = {
        "w_vin": np.ascontiguousarray(v_in_w.T).astype(NPBF),
        "w_hq": np.ascontiguousarray(h_in_w[0:E].T).astype(NPBF),
        "w_hkv": np.ascontiguousarray(h_in_w[E:3 * E].T).astype(NPBF),
        "w_fh": np.ascontiguousarray(Wfh.T).astype(NPBF),
        "w_fv": np.ascontiguousarray(Wfv.T).astype(NPBF),
        "w_m2": np.ascontiguousarray(mlp_w2.T).astype(NPBF),
        "mask": _band_masks(),
        "biases": biases,
    }

    in_maps = []
    for c in range(NCORE):
        rows = x[RPC * c:RPC * (c + 1)]
        cols = x[:, RPC * c:RPC * (c + 1)].transpose(1, 0, 2)
        m = dict(shared)
        m["xr_t"] = np.ascontiguousarray(rows.reshape(T, E).T).astype(NPBF)
        m["xc_t"] = np.ascontiguousarray(cols.reshape(T, E).T).astype(NPBF)
        in_maps.append(m)

    global _LAST_IN_MAPS
    _LAST_IN_MAPS = in_maps
    res = run_bass_kernel_spmd(nc, in_maps, core_ids=list(range(NCORE)))

    out = np.empty((S, S, E), np.float32)
    for c in range(NCORE):
        out[RPC * c:RPC * (c + 1)] = res.results[c]["out_t"].astype(np.float32).T.reshape(RPC, S, E)
    return out



# revision 53
# speedup vs baseline: 1.1813x; 1.1542x over previous
"""BiSPA (bidirectional sparse windowed attention + MLP) Trainium2 kernel.

Full inputs in, full outputs out; core c owns output rows [24c, 24c+24).
Optimized v9 (~458-465us at full clock vs 511us v8 / 1065us baseline;
P0 throttle phases add up to ~10%). Key ideas on top of v8:
- feature-major turn (ctx token-major -> MLP feature-major) moved off the
  PE: XBAR DMA-transposes on the sync queue ([96,1024] -> [128,8,96] into
  a single per-group ct tile), replacing 32 PE transposes + 8 DVE copies
  per group. PSUM freed by pcx -> psS double-buffered.
- softmax normalization as stride-0-broadcast DVE tensor_tensor
  ([96, nh, 64] ctx times recip[96, nh, 1]): 2 ops per attn@V bank
  instead of 2*nh per-head muls split across DVE/ScalarE (ScalarE busy
  68% -> 45%).
- score pairs packed 2-per-[128,1024] tile with h2=0/h2=1 blocks in
  separate banks: quad row-tiles drain to distinct PSUM banks, so each
  bank must stay row-group pure (mixing row groups in a bank hangs the
  NEFF). Frees 2 banks -> psA triple-buffered.
- final group's MLP split per strip (token columns are independent):
  strip-0 half runs while strip-1 attention/transposes finish.
- startup: weights on the sync queue in consumption order (whq, whkv,
  wv-QK | wv-V split), xr before xc; first matmul at ~10us.
- attn@V ones-columns written once (group 0 touches all 8 pv buffers).
- output DMA'd as bf16 (half the bytes), host converts to fp32.
Known limits: scores/attn@V are LDWEIGHTS-port-bound (~80-90ns/matmul vs
40ns stream); fp8 fails the 2e-2 gate (V-proj/MLP2 ~3.4-4.3e-2 alone);
ctx^T attn@V dead-ends on partition-broadcast for Z (DVE lanes are
partition-locked, gpsimd ops cost ~14.5us each).
"""

import numpy as np
from contextlib import ExitStack

import concourse.bass as bass
import concourse.mybir as mybir
import concourse.tile as tile
from concourse import bacc
from concourse.bass_utils import run_bass_kernel_spmd
from concourse.tile import add_dep_helper


def _chain(insts):
    for a, b in zip(insts, insts[1:]):
        add_dep_helper(b.ins, a.ins, sync=False, reason="psum-bank group order")

BF = mybir.dt.bfloat16
F32 = mybir.dt.float32
AF = mybir.ActivationFunctionType
MUL = mybir.AluOpType.mult
NPBF = mybir.dt.np(BF)

E = 512
H = 8
D = 64
W = 32
S = 192
NCORE = 8
RPC = 24
T = RPC * S


def _band_masks():
    """Score mask, bf16 (128, 384): [TA 96 | TB 96] x 2 heads.

    q-blocks split at 96 so neither straddles a key block:
      TA: rows k in [0,128), cols q in [0,96):    valid = |k-q| <= W
      TB: rows k = 64+r in [64,192), cols q = 96+c in [96,192):
          valid = |k-q| <= W
    """
    k = np.arange(128)[:, None]
    qa = np.arange(96)[None, :]
    ta = (np.abs(k - qa) <= W)
    kb = 64 + np.arange(128)[:, None]
    qb = 96 + np.arange(96)[None, :]
    tb = (np.abs(kb - qb) <= W)
    m = np.concatenate([ta, tb], axis=1).astype(np.float32)
    return np.concatenate([m, m], axis=1).astype(NPBF)


def _build_program(bias_flags):
    has_vqk_b, has_hq_b, has_hk_b, has_beff, has_b2 = bias_flags

    nc = bacc.Bacc("TRN2", target_bir_lowering=False, debug=False,
                   num_devices=NCORE, num_swdge_queues=4)

    xr_t = nc.dram_tensor("xr_t", [E, T], BF, kind="ExternalInput").ap()
    xc_t = nc.dram_tensor("xc_t", [E, T], BF, kind="ExternalInput").ap()
    w_vin = nc.dram_tensor("w_vin", [E, 3 * E], BF, kind="ExternalInput").ap()
    w_hq = nc.dram_tensor("w_hq", [E, E], BF, kind="ExternalInput").ap()
    w_hkv = nc.dram_tensor("w_hkv", [E, 2 * E], BF, kind="ExternalInput").ap()
    w_fh = nc.dram_tensor("w_fh", [E, E], BF, kind="ExternalInput").ap()
    w_fv = nc.dram_tensor("w_fv", [E, E], BF, kind="ExternalInput").ap()
    w_m2 = nc.dram_tensor("w_m2", [E, E], BF, kind="ExternalInput").ap()
    mask_d = nc.dram_tensor("mask", [128, 384], BF, kind="ExternalInput").ap()
    bias_d = nc.dram_tensor("biases", [128, 24], F32, kind="ExternalInput").ap()
    out_t = nc.dram_tensor("out_t", [E, T], BF, kind="ExternalOutput").ap()

    with tile.TileContext(nc) as tc, ExitStack() as ctx:
        pw = ctx.enter_context(tc.tile_pool(name="pw", bufs=1))
        psA = ctx.enter_context(tc.tile_pool(name="psA", bufs=4, space="PSUM"))
        psS = ctx.enter_context(tc.tile_pool(name="psS", bufs=1, space="PSUM"))
        psC = ctx.enter_context(tc.tile_pool(name="psC", bufs=2, space="PSUM"))
        px = ctx.enter_context(tc.tile_pool(name="px", bufs=3))
        pqk = ctx.enter_context(tc.tile_pool(name="pqk", bufs=32))
        pv = ctx.enter_context(tc.tile_pool(name="pv", bufs=8))
        pp = ctx.enter_context(tc.tile_pool(name="pp", bufs=20))
        pzr = ctx.enter_context(tc.tile_pool(name="pzr", bufs=12))
        pct = ctx.enter_context(tc.tile_pool(name="pct", bufs=3))
        phid = ctx.enter_context(tc.tile_pool(name="phid", bufs=8))
        pout = ctx.enter_context(tc.tile_pool(name="pout", bufs=8))

        def load_const(name, dram_ap, shape, dtype, eng=None):
            # weights split across the sync/scalar queues: the ~600ns issue
            # cost per DMA dominates startup if serialized on one queue
            t = pw.tile(shape, dtype, tag=name)
            (eng or nc.sync).dma_start(t[:], dram_ap)
            return t

        import os as _os
        NPAIR = int(_os.environ.get("BISPA_NPAIRS", RPC // 2))

        def load_x(g):
            # all xr tiles first: the group's first matmuls (h-branch Q)
            # need only xr; xc is first touched at j=4
            g0 = 2 * S * g
            xr2, xc2 = [], []
            for k in range(4):
                t = px.tile([128, 2 * S], BF, tag=f"xr{k}", name=f"xr{k}_{g}")
                nc.gpsimd.dma_start(t[:], xr_t[128 * k:128 * (k + 1), g0:g0 + 2 * S])
                xr2.append(t)
            for k in range(4):
                t = px.tile([128, 2 * S], BF, tag=f"xc{k}", name=f"xc{k}_{g}")
                nc.gpsimd.dma_start(t[:], xc_t[128 * k:128 * (k + 1), g0:g0 + 2 * S])
                xc2.append(t)
            return xr2, xc2

        xnext = load_x(0)

        # load order = consumption order: group-0 x tiles are queued first
        # (see load_x below); the first emitted matmuls are the h-branch QK
        # projections (whq, whkv), then v (wv), then attention constants,
        # then the MLP weights which are first needed one group later.
        whq = [load_const(f"whq{k}", w_hq[128 * k:128 * (k + 1), :], [128, E], BF)
               for k in range(4)]
        whkv = [load_const(f"whkv{k}", w_hkv[128 * k:128 * (k + 1), :], [128, 2 * E], BF)
                for k in range(4)]
        # wv split: QK columns land first (group-0 j-loop), V columns later
        wv = []
        for k in range(4):
            t = pw.tile([128, 3 * E], BF, tag=f"wv{k}")
            nc.sync.dma_start(t[:, 0:2 * E], w_vin[128 * k:128 * (k + 1), 0:2 * E])
            wv.append(t)
        for k in range(4):
            nc.sync.dma_start(wv[k][:, 2 * E:3 * E],
                              w_vin[128 * k:128 * (k + 1), 2 * E:3 * E])
        msk = load_const("msk", mask_d[:, :], [128, 384], BF)
        bia = load_const("bia", bias_d[:, :], [128, 24], F32)
        wfh = [load_const(f"wfh{k}", w_fh[128 * k:128 * (k + 1), :], [128, E], BF)
               for k in range(4)]
        wfv = [load_const(f"wfv{k}", w_fv[128 * k:128 * (k + 1), :], [128, E], BF)
               for k in range(4)]
        wm2 = [load_const(f"wm2{k}", w_m2[128 * k:128 * (k + 1), :], [128, E], BF)
               for k in range(4)]

        # bias cols: 0-7 v_in QK; 8-11 h Q; 12-15 h K; 16-19 b_eff; 20-23 b2

        prev_mlp = []   # deferred MLP closures from the previous group
        for g in range(NPAIR):
            g0 = 2 * S * g
            xr2, xc2 = xnext

            # ---------- QK projections, feature-major, N=384 ----------
            # emitted in score-consumption order: pair p's Q (j=p) and K
            # (j=4+p) for both branches land before pair p+1's, so pair-0
            # scores never wait on late evictions
            qk = {"h": [None] * 8, "v": [None] * 8}
            for p in range(4):
                for j in (p, 4 + p):
                    for br in ("h", "v"):
                        ps = psA.tile([128, 384], F32, tag="proj",
                                      padded_shape=[128, 512])
                        for k in range(4):
                            if br == "v":
                                lhsT = wv[k][:, 128 * j:128 * (j + 1)]
                                rhs = xr2[k][:]
                            elif j < 4:
                                lhsT = whq[k][:, 128 * j:128 * (j + 1)]
                                rhs = xr2[k][:]
                            else:
                                lhsT = whkv[k][:, 128 * (j - 4):128 * (j - 3)]
                                rhs = xc2[k][:]
                            nc.tensor.matmul(ps[:], lhsT=lhsT, rhs=rhs,
                                             start=(k == 0), stop=(k == 3))
                        bcol = j if br == "v" else (8 + j)
                        has_b = ((has_vqk_b and br == "v")
                                 or (has_hq_b and br == "h" and j < 4)
                                 or (has_hk_b and br == "h" and j >= 4))
                        dst = pqk.tile([128, 384], BF, tag="qk")
                        if has_b:
                            nc.scalar.activation(dst[:], ps[:], AF.Identity,
                                                 bias=bia[:, bcol:bcol + 1])
                        else:
                            # no bias: split evictions across engines by br
                            if br == "h":
                                nc.vector.tensor_copy(dst[:], ps[:])
                            else:
                                nc.scalar.activation(dst[:], ps[:], AF.Identity)
                        qk[br][j] = dst

            if g + 1 < NPAIR:
                xnext = load_x(g + 1)

            # ct: [128 f, 8 f-blocks (0-3 h, 4-7 v), 2S tokens] single tile
            # so the feature-major turn is 2 XBAR DMA-transposes per strip.
            # Final group: one tile per strip instead — DMA-transpose write
            # tracking is tile-granular, and the strip-0 MLP half must not
            # wait on strip-1's transposes.
            if g + 1 < NPAIR:
                ct = pct.tile([128, 8, 2 * S], BF, tag="ct", name=f"ct_{g}")
                cts = [ct]
            else:
                cts = [pct.tile([128, 8, S], BF, tag=f"ctf{a}", name=f"ctf_{a}")
                       for a in range(2)]
                ct = None

            for a in range(2):
                s0 = S * a
                # ---- V projections for both branches first ----
                vab = {}
                for br in ("h", "v"):
                    xin = xr2 if br == "v" else xc2
                    vcols = slice(1024, 1536) if br == "v" else slice(512, 1024)
                    vw = wv if br == "v" else whkv
                    vps_a = psA.tile([128, 512], F32, tag="proj")
                    for k in range(4):
                        nc.tensor.matmul(vps_a[:], lhsT=xin[k][:, s0:s0 + 128],
                                         rhs=vw[k][:, vcols],
                                         start=(k == 0), stop=(k == 3))
                    vps_b = psA.tile([128, 512], F32, tag="proj")
                    for k in range(4):
                        nc.tensor.matmul(vps_b[:], lhsT=xin[k][:, s0 + 64:s0 + 192],
                                         rhs=vw[k][:, vcols],
                                         start=(k == 0), stop=(k == 3))
                    va = pv.tile([128, 8, 65], BF, tag="vp")
                    vb = pv.tile([128, 8, 65], BF, tag="vp")
                    nc.vector.tensor_copy(
                        va[:, :, 0:64],
                        vps_a[:].rearrange("p (h c) -> p h c", c=64))
                    nc.vector.tensor_copy(
                        vb[:, :, 0:64],
                        vps_b[:].rearrange("p (h c) -> p h c", c=64))
                    if g == 0:
                        # ones columns: group 0's 8 allocs touch all 8 pv
                        # buffers once; later strips only rewrite [:, :, 0:64]
                        nc.vector.memset(va[:, :, 64:65], 1.0)
                        nc.vector.memset(vb[:, :, 64:65], 1.0)
                    vab[br] = (va, vb)

                # ---- scores + exp + mask, pair-steps interleaved h/v ----
                # pm[br][p]: (128, 448) masked probs for heads 2p, 2p+1
                pm = {"h": [None] * 4, "v": [None] * 4}
                # attn@V bank plan: per br, 3 psC tiles:
                #   A: heads 0-2, B: heads 3-5, C: heads 6-7
                cxt = {"h": [None] * 3, "v": [None] * 3}
                zrs = {"h": [None] * 3, "v": [None] * 3}
                # normalized ctx, token-major: [96 q', 2 (h|v), 512 f] so one
                # XBAR transpose covers both branches
                ctxn1 = pp.tile([96, 2, 512], BF, tag="ctxn1", bufs=4)
                ctxn2 = pp.tile([96, 2, 512], BF, tag="ctxn2", bufs=4)
                BI = {"h": 0, "v": 1}

                def emit_pair(br, p, spt=None):
                    # both branches of a pair-step share one [128,1024] tile:
                    # bank A (cols 0:512) holds the h2=0 blocks of h and v,
                    # bank B the h2=1 blocks, keeping each PSUM bank row-
                    # group pure (quad row-tiles drain to distinct banks)
                    boff = 0 if br == "h" else 192
                    QT = qk[br][p][:, s0:s0 + S]
                    KT = qk[br][4 + p][:, s0:s0 + S]
                    mmsc = []
                    for h2 in range(2):
                        d0 = 64 * h2
                        c0 = 512 * h2 + boff
                        mmsc.append(nc.tensor.matmul(
                            spt[:, c0:c0 + 96],
                            lhsT=KT[d0:d0 + 64, 0:128],
                            rhs=QT[d0:d0 + 64, 0:96],
                            start=True, stop=True, skip_group_check=True))
                        mmsc.append(nc.tensor.matmul(
                            spt[:, c0 + 96:c0 + 192],
                            lhsT=KT[d0:d0 + 64, 64:192],
                            rhs=QT[d0:d0 + 64, 96:192],
                            start=True, stop=True, skip_group_check=True))
                    pb = pp.tile([128, 384], BF, tag="p")
                    sin = spt[:].rearrange("p (b c) -> p b c", c=512)[:, :, boff:boff + 192]
                    ex = nc.scalar.activation(
                        pb[:].rearrange("p (b c) -> p b c", c=192),
                        sin, AF.Exp, scale=0.125)
                    for m in mmsc:
                        add_dep_helper(ex.ins, m.ins, sync=True,
                                       reason="exp after score mms")
                    pmt = pp.tile([128, 384], BF, tag="p")
                    nc.vector.tensor_tensor(pmt[:], pb[:], msk[:], op=MUL)
                    pm[br][p] = pmt

                def bank_mms(br, b):
                    """attn@V bank b: heads hs = 3b..3b+2 (bank 2: h6,h7).
                    Per head 2 matmuls: q' [0,96) from TA keys [0,128) (va),
                    q' [96,192) from TB keys [64,192) (vb). Output rows 0:96,
                    head i at cols [130i, 130i+130) = [q1' 65 | q2' 65]."""
                    hs = [3 * b + i for i in range(3 if b < 2 else 2)]
                    va, vb = vab[br]
                    cp = psC.tile([128, 130 * len(hs)], F32, tag="cx",
                                  padded_shape=[128, 512], name=f"cp_{br}_{b}")
                    mms = []
                    n = 2 * len(hs)
                    for i, h in enumerate(hs):
                        pmt = pm[br][h // 2]
                        ta = 192 * (h % 2)
                        cb = 130 * i
                        mms.append(lambda i=i, h=h, pmt=pmt, ta=ta, cb=cb: nc.tensor.matmul(
                            cp[0:96, cb:cb + 65], lhsT=pmt[:, ta:ta + 96],
                            rhs=va[:, h:h + 1, :], start=(2 * i == 0),
                            stop=(2 * i == n - 1), skip_group_check=True))
                        mms.append(lambda i=i, h=h, pmt=pmt, ta=ta, cb=cb: nc.tensor.matmul(
                            cp[0:96, cb + 65:cb + 130],
                            lhsT=pmt[:, ta + 96:ta + 192],
                            rhs=vb[:, h:h + 1, :], start=(2 * i + 1 == 0),
                            stop=(2 * i + 1 == n - 1), skip_group_check=True))
                    return cp, mms, hs

                def emit_banks(b):
                    """Emit h and v banks with matmuls interleaved so the
                    per-matmul SBUF latency of one bank's chain overlaps the
                    other bank's execution (different PSUM banks)."""
                    cph, mmh, hs = bank_mms("h", b)
                    cpv, mmv, _ = bank_mms("v", b)
                    outh, outv = [], []
                    for fh, fv in zip(mmh, mmv):
                        outh.append(fh())
                        outv.append(fv())
                    _chain(outh)
                    _chain(outv)
                    cxt["h"][b] = (cph, outh[-1], hs)
                    cxt["v"][b] = (cpv, outv[-1], hs)

                def emit_norm(br, b):
                    # normalize: one reciprocal + two broadcast tensor_tensor
                    # ops per bank ([96, nh, 64] x stride-0 recip) instead of
                    # 2*nh small per-head muls split across DVE/ScalarE
                    cp, lastmm, hs = cxt[br][b]
                    nh = len(hs)
                    h0 = hs[0]
                    zr = pzr.tile([96, 2, nh, 1], F32, tag="zr")
                    # Z columns viewed (qblock, head): [96, 2, nh, 1]
                    cpz = cp[0:96, 0:130 * nh].rearrange(
                        "p (h q c) -> p q h c", q=2, c=65)
                    reads = [nc.vector.reciprocal(zr[:], cpz[:, :, :, 64:65])]
                    cph = cp[0:96, 0:130 * nh].rearrange("p (x c) -> p x c", c=130)
                    for qb, dst in ((0, ctxn1), (1, ctxn2)):
                        in0 = cph[:, :, 65 * qb:65 * qb + 64]
                        b0, b1 = bass.broadcast_tensor_aps(in0, zr[:, qb, :, :])
                        out = dst[0:96, BI[br], 64 * h0:64 * (h0 + nh)]
                        reads.append(nc.vector.tensor_tensor(
                            out.rearrange("p (x c) -> p x c", c=64),
                            b0, b1, op=MUL))
                    for r in reads:
                        add_dep_helper(r.ins, lastmm.ins, sync=True,
                                       reason="psum read after group close")

                # emission: pair-steps with attn@V banks interleaved; the
                # previous group's MLP units are sprinkled between steps so
                # the in-order PE always has a ready big matmul to chew on
                # skip early sites: strip 0 waits out the previous group's
                # in-flight ct transposes; strip 1 defers so the last units
                # cover the late-strip banks2/norm2 stalls (12 units, 16
                # sites -> pops at strip-0 sites 2-8, strip-1 sites 4-8)
                skip = [1 if a == 0 else 3]

                def mlp_step():
                    if skip[0] > 0:
                        skip[0] -= 1
                        return
                    if prev_mlp:
                        prev_mlp.pop(0)()

                spt0 = psS.tile([128, 1024], F32, tag="sc")
                for br in ("h", "v"):
                    emit_pair(br, 0, spt0)
                mlp_step()
                spt1 = psS.tile([128, 1024], F32, tag="sc")
                for br in ("h", "v"):
                    emit_pair(br, 1, spt1)
                mlp_step()
                emit_banks(0)             # heads 0-2 (needs pairs 0,1)
                mlp_step()
                spt2 = psS.tile([128, 1024], F32, tag="sc")
                for br in ("h", "v"):
                    emit_pair(br, 2, spt2)
                mlp_step()
                for br in ("h", "v"):
                    emit_norm(br, 0)
                mlp_step()
                emit_banks(1)             # heads 3-5 (needs pairs 1,2)
                mlp_step()
                spt3 = psS.tile([128, 1024], F32, tag="sc")
                for br in ("h", "v"):
                    emit_pair(br, 3, spt3)
                mlp_step()
                for br in ("h", "v"):
                    emit_norm(br, 1)
                emit_banks(2)             # heads 6,7
                mlp_step()
                for br in ("h", "v"):
                    emit_norm(br, 2)

                # ---- feature-major turn via XBAR DMA transpose: one call
                # per (strip, q-block) [96,1024] -> [128, 8, 96]; the
                # ~1.3us cost per call sits on the idle sync queue ----
                ctd = cts[0] if g + 1 < NPAIR else cts[a]
                c0 = s0 if g + 1 < NPAIR else 0
                nc.sync.dma_start_transpose(
                    ctd[:, :, c0:c0 + 96], ctxn1[:, :, :])
                nc.sync.dma_start_transpose(
                    ctd[:, :, c0 + 96:c0 + 192], ctxn2[:, :, :])

            # ---------- fused out-proj + MLP1 + MLP2 as deferred closures,
            # emitted interleaved into the NEXT group's attention ----------
            def build_mlp(ct=cts[0], g0=g0):
                units = []
                hid = []

                psj = {}

                def hid_unit_a(j):
                    ps = psA.tile([128, 384], F32, tag="proj",
                                  padded_shape=[128, 512], name=f"mlp1_{j}")
                    psj[j] = ps
                    for k in range(4):
                        nc.tensor.matmul(ps[:],
                                         lhsT=wfh[k][:, 128 * j:128 * (j + 1)],
                                         rhs=ct[:, k, :],
                                         start=(k == 0), stop=False)

                def hid_unit_b(j):
                    ps = psj[j]
                    for k in range(4):
                        nc.tensor.matmul(ps[:],
                                         lhsT=wfv[k][:, 128 * j:128 * (j + 1)],
                                         rhs=ct[:, 4 + k, :],
                                         start=False, stop=(k == 3))
                    dst = phid.tile([128, 384], BF, tag="hid", name=f"hid_{j}")
                    if has_beff:
                        nc.scalar.activation(dst[:], ps[:], AF.Relu,
                                             bias=bia[:, 16 + j:16 + j + 1])
                    else:
                        nc.scalar.activation(dst[:], ps[:], AF.Relu)
                    hid.append(dst)

                def out_unit(j):
                    ps = psA.tile([128, 384], F32, tag="proj",
                                  padded_shape=[128, 512], name=f"mlp2_{j}")
                    for k in range(4):
                        nc.tensor.matmul(ps[:],
                                         lhsT=wm2[k][:, 128 * j:128 * (j + 1)],
                                         rhs=hid[k][:],
                                         start=(k == 0), stop=(k == 3))
                    osb = pout.tile([128, 384], BF, tag="o", name=f"osb_{j}")
                    if has_b2:
                        nc.scalar.activation(osb[:], ps[:], AF.Identity,
                                             bias=bia[:, 20 + j:20 + j + 1])
                    else:
                        nc.scalar.activation(osb[:], ps[:], AF.Identity)
                    nc.sync.dma_start(
                        out_t[128 * j:128 * (j + 1), g0:g0 + 2 * S], osb[:])

                for j in range(4):
                    units.append(lambda j=j: hid_unit_a(j))
                    units.append(lambda j=j: hid_unit_b(j))
                for j in range(4):
                    units.append(lambda j=j: out_unit(j))
                return units

            def build_mlp_final(cts=cts, g0=g0):
                """Final group: token columns are independent through the MLP
                (contraction is over features), so strip-0's half runs as
                self-contained units during strip-1's attention; only the
                strip-1 half waits on the last ct transposes."""
                hid = {}

                def get_hid(j):
                    if j not in hid:
                        hid[j] = phid.tile([128, 384], BF, tag="hid",
                                           name=f"hidF_{j}")
                    return hid[j]

                def full_unit(j, a):
                    cta = cts[a]
                    c0 = S * a
                    ps = psA.tile([128, 384], F32, tag="proj",
                                  padded_shape=[128, 512], name=f"mlpF_{j}_{a}")
                    for k in range(4):
                        nc.tensor.matmul(ps[:, 0:S],
                                         lhsT=wfh[k][:, 128 * j:128 * (j + 1)],
                                         rhs=cta[:, k, :],
                                         start=(k == 0), stop=False)
                    for k in range(4):
                        nc.tensor.matmul(ps[:, 0:S],
                                         lhsT=wfv[k][:, 128 * j:128 * (j + 1)],
                                         rhs=cta[:, 4 + k, :],
                                         start=False, stop=(k == 3))
                    dst = get_hid(j)
                    if has_beff:
                        nc.scalar.activation(dst[:, c0:c0 + S], ps[:, 0:S],
                                             AF.Relu,
                                             bias=bia[:, 16 + j:16 + j + 1])
                    else:
                        nc.scalar.activation(dst[:, c0:c0 + S], ps[:, 0:S],
                                             AF.Relu)

                def out_unit(j):
                    ps = psA.tile([128, 384], F32, tag="proj",
                                  padded_shape=[128, 512], name=f"mlp2F_{j}")
                    for k in range(4):
                        nc.tensor.matmul(ps[:],
                                         lhsT=wm2[k][:, 128 * j:128 * (j + 1)],
                                         rhs=get_hid(k)[:],
                                         start=(k == 0), stop=(k == 3))
                    osb = pout.tile([128, 384], BF, tag="o", name=f"osbF_{j}")
                    if has_b2:
                        nc.scalar.activation(osb[:], ps[:], AF.Identity,
                                             bias=bia[:, 20 + j:20 + j + 1])
                    else:
                        nc.scalar.activation(osb[:], ps[:], AF.Identity)
                    nc.sync.dma_start(
                        out_t[128 * j:128 * (j + 1), g0:g0 + 2 * S], osb[:])

                u0 = [lambda j=j: full_unit(j, 0) for j in range(4)]
                u1 = [lambda j=j: full_unit(j, 1) for j in range(4)]
                u1 += [lambda j=j: out_unit(j) for j in range(4)]
                return u0, u1

            for f in prev_mlp:   # drain any leftovers (shouldn't happen)
                f()
            if g + 1 < NPAIR:
                prev_mlp = build_mlp()
            else:
                fin_u0, fin_u1 = build_mlp_final()
                prev_mlp = fin_u0 + fin_u1

        for f in prev_mlp:       # final group's MLP
            f()
    nc.finalize()
    return nc


_CACHE = {}


def _get_program(bias_flags):
    key = tuple(bias_flags)
    if key not in _CACHE:
        _CACHE[key] = _build_program(key)
    return _CACHE[key]


def _col(b):
    return np.ascontiguousarray(b.reshape(-1, 128).T.astype(np.float32))


def kernel(hidden_states, h_in_w, h_in_b, h_out_w, h_out_b,
           v_in_w, v_in_b, v_out_w, v_out_b,
           mlp_w1, mlp_b1, mlp_w2, mlp_b2):
    x = np.asarray(hidden_states, dtype=np.float32)
    h_in_w = np.asarray(h_in_w, np.float32)
    h_in_b = np.asarray(h_in_b, np.float32)
    h_out_w = np.asarray(h_out_w, np.float32)
    h_out_b = np.asarray(h_out_b, np.float32)
    v_in_w = np.asarray(v_in_w, np.float32)
    v_in_b = np.asarray(v_in_b, np.float32)
    v_out_w = np.asarray(v_out_w, np.float32)
    v_out_b = np.asarray(v_out_b, np.float32)
    mlp_w1 = np.asarray(mlp_w1, np.float32)
    mlp_b1 = np.asarray(mlp_b1, np.float32)
    mlp_w2 = np.asarray(mlp_w2, np.float32)
    mlp_b2 = np.asarray(mlp_b2, np.float32)

    # V biases shift ctx by a constant (softmax weights sum to 1): fold through
    # out-proj; then fold out-proj entirely into MLP1 (relu is the only
    # nonlinearity after it): hid = relu(h_ctx@Wfh^T + v_ctx@Wfv^T + b_eff).
    h_out_eff = h_out_b + h_out_w @ h_in_b[2 * E:3 * E]
    v_out_eff = v_out_b + v_out_w @ v_in_b[2 * E:3 * E]
    W1h = mlp_w1[:, 0:E]
    W1v = mlp_w1[:, E:2 * E]
    Wfh = W1h @ h_out_w            # (E, E)
    Wfv = W1v @ v_out_w
    b_eff = mlp_b1 + W1h @ h_out_eff + W1v @ v_out_eff

    bias_flags = (
        bool(np.any(v_in_b[0:2 * E])), bool(np.any(h_in_b[0:E])),
        bool(np.any(h_in_b[E:2 * E])), bool(np.any(b_eff)),
        bool(np.any(mlp_b2)),
    )
    nc = _get_program(bias_flags)

    biases = np.zeros((128, 24), np.float32)
    biases[:, 0:8] = _col(v_in_b[0:2 * E])
    biases[:, 8:16] = _col(h_in_b[0:2 * E])
    biases[:, 16:20] = _col(b_eff)
    biases[:, 20:24] = _col(mlp_b2)

    shared = {
        "w_vin": np.ascontiguousarray(v_in_w.T).astype(NPBF),
        "w_hq": np.ascontiguousarray(h_in_w[0:E].T).astype(NPBF),
        "w_hkv": np.ascontiguousarray(h_in_w[E:3 * E].T).astype(NPBF),
        "w_fh": np.ascontiguousarray(Wfh.T).astype(NPBF),
        "w_fv": np.ascontiguousarray(Wfv.T).astype(NPBF),
        "w_m2": np.ascontiguousarray(mlp_w2.T).astype(NPBF),
        "mask": _band_masks(),
        "biases": biases,
    }

    in_maps = []
    for c in range(NCORE):
        rows = x[RPC * c:RPC * (c + 1)]
        cols = x[:, RPC * c:RPC * (c + 1)].transpose(1, 0, 2)
        m = dict(shared)
        m["xr_t"] = np.ascontiguousarray(rows.reshape(T, E).T).astype(NPBF)
        m["xc_t"] = np.ascontiguousarray(cols.reshape(T, E).T).astype(NPBF)
        in_maps.append(m)

    global _LAST_IN_MAPS
    _LAST_IN_MAPS = in_maps
    res = run_bass_kernel_spmd(nc, in_maps, core_ids=list(range(NCORE)))

    out = np.empty((S, S, E), np.float32)
    for c in range(NCORE):
        out[RPC * c:RPC * (c + 1)] = res.results[c]["out_t"].astype(np.float32).T.reshape(RPC, S, E)
    return out



# revision 54
# speedup vs baseline: 1.1903x; 1.0077x over previous
"""BiSPA (bidirectional sparse windowed attention + MLP) Trainium2 kernel.

Full inputs in, full outputs out; core c owns output rows [24c, 24c+24).
Optimized v9 (~457-460us at full clock vs 511us v8 / 1065us baseline;
P0 throttle phases add up to ~10%). Key ideas on top of v8:
- feature-major turn (ctx token-major -> MLP feature-major) moved off the
  PE: XBAR DMA-transposes on the sync queue ([96,1024] -> [128,8,96] into
  a single per-group ct tile), replacing 32 PE transposes + 8 DVE copies
  per group. PSUM freed by pcx -> psS double-buffered.
- softmax normalization as stride-0-broadcast DVE tensor_tensor
  ([96, nh, 64] ctx times recip[96, nh, 1]): 2 ops per attn@V bank
  instead of 2*nh per-head muls split across DVE/ScalarE (ScalarE busy
  68% -> 45%).
- score pairs packed 2-per-[128,1024] tile with h2=0/h2=1 blocks in
  separate banks: quad row-tiles drain to distinct PSUM banks, so each
  bank must stay row-group pure (mixing row groups in a bank hangs the
  NEFF). Frees 2 banks -> psA quadruple-buffered (8/8 banks used).
- QK projections emitted in score-consumption order (pair p's Q and K
  for both branches before pair p+1): kills the score-waits-on-eviction
  stalls (S163 DVE-wait gaps -> 0).
- final group's MLP split per strip (token columns are independent):
  strip-0 half runs while strip-1 attention/transposes finish.
- startup: weights on the sync queue in consumption order (whq, whkv,
  wv-QK | wv-V split), xr before xc; first matmul at ~10us.
- attn@V ones-columns written once (group 0 touches all 8 pv buffers).
- output DMA'd as bf16 (half the bytes), host converts to fp32.
Known limits: scores/attn@V are LDWEIGHTS-port-bound (~80-90ns/matmul vs
40ns stream); fp8 fails the 2e-2 gate (V-proj/MLP2 ~3.4-4.3e-2 alone);
ctx^T attn@V dead-ends on partition-broadcast for Z (DVE lanes are
partition-locked, gpsimd ops cost ~14.5us each).
"""

import numpy as np
from contextlib import ExitStack

import concourse.bass as bass
import concourse.mybir as mybir
import concourse.tile as tile
from concourse import bacc
from concourse.bass_utils import run_bass_kernel_spmd
from concourse.tile import add_dep_helper


def _chain(insts):
    for a, b in zip(insts, insts[1:]):
        add_dep_helper(b.ins, a.ins, sync=False, reason="psum-bank group order")

BF = mybir.dt.bfloat16
F32 = mybir.dt.float32
AF = mybir.ActivationFunctionType
MUL = mybir.AluOpType.mult
NPBF = mybir.dt.np(BF)

E = 512
H = 8
D = 64
W = 32
S = 192
NCORE = 8
RPC = 24
T = RPC * S


def _band_masks():
    """Score mask, bf16 (128, 384): [TA 96 | TB 96] x 2 heads.

    q-blocks split at 96 so neither straddles a key block:
      TA: rows k in [0,128), cols q in [0,96):    valid = |k-q| <= W
      TB: rows k = 64+r in [64,192), cols q = 96+c in [96,192):
          valid = |k-q| <= W
    """
    k = np.arange(128)[:, None]
    qa = np.arange(96)[None, :]
    ta = (np.abs(k - qa) <= W)
    kb = 64 + np.arange(128)[:, None]
    qb = 96 + np.arange(96)[None, :]
    tb = (np.abs(kb - qb) <= W)
    m = np.concatenate([ta, tb], axis=1).astype(np.float32)
    return np.concatenate([m, m], axis=1).astype(NPBF)


def _build_program(bias_flags):
    has_vqk_b, has_hq_b, has_hk_b, has_beff, has_b2 = bias_flags

    nc = bacc.Bacc("TRN2", target_bir_lowering=False, debug=False,
                   num_devices=NCORE, num_swdge_queues=4)

    xr_t = nc.dram_tensor("xr_t", [E, T], BF, kind="ExternalInput").ap()
    xc_t = nc.dram_tensor("xc_t", [E, T], BF, kind="ExternalInput").ap()
    w_vin = nc.dram_tensor("w_vin", [E, 3 * E], BF, kind="ExternalInput").ap()
    w_hq = nc.dram_tensor("w_hq", [E, E], BF, kind="ExternalInput").ap()
    w_hkv = nc.dram_tensor("w_hkv", [E, 2 * E], BF, kind="ExternalInput").ap()
    w_fh = nc.dram_tensor("w_fh", [E, E], BF, kind="ExternalInput").ap()
    w_fv = nc.dram_tensor("w_fv", [E, E], BF, kind="ExternalInput").ap()
    w_m2 = nc.dram_tensor("w_m2", [E, E], BF, kind="ExternalInput").ap()
    mask_d = nc.dram_tensor("mask", [128, 384], BF, kind="ExternalInput").ap()
    bias_d = nc.dram_tensor("biases", [128, 24], F32, kind="ExternalInput").ap()
    out_t = nc.dram_tensor("out_t", [E, T], BF, kind="ExternalOutput").ap()

    with tile.TileContext(nc) as tc, ExitStack() as ctx:
        pw = ctx.enter_context(tc.tile_pool(name="pw", bufs=1))
        psA = ctx.enter_context(tc.tile_pool(name="psA", bufs=4, space="PSUM"))
        psS = ctx.enter_context(tc.tile_pool(name="psS", bufs=1, space="PSUM"))
        psC = ctx.enter_context(tc.tile_pool(name="psC", bufs=2, space="PSUM"))
        px = ctx.enter_context(tc.tile_pool(name="px", bufs=3))
        pqk = ctx.enter_context(tc.tile_pool(name="pqk", bufs=32))
        pv = ctx.enter_context(tc.tile_pool(name="pv", bufs=8))
        pp = ctx.enter_context(tc.tile_pool(name="pp", bufs=20))
        pzr = ctx.enter_context(tc.tile_pool(name="pzr", bufs=12))
        pct = ctx.enter_context(tc.tile_pool(name="pct", bufs=3))
        phid = ctx.enter_context(tc.tile_pool(name="phid", bufs=8))
        pout = ctx.enter_context(tc.tile_pool(name="pout", bufs=8))

        def load_const(name, dram_ap, shape, dtype, eng=None):
            # weights split across the sync/scalar queues: the ~600ns issue
            # cost per DMA dominates startup if serialized on one queue
            t = pw.tile(shape, dtype, tag=name)
            (eng or nc.sync).dma_start(t[:], dram_ap)
            return t

        import os as _os
        NPAIR = int(_os.environ.get("BISPA_NPAIRS", RPC // 2))

        def load_x(g):
            # all xr tiles first: the group's first matmuls (h-branch Q)
            # need only xr; xc is first touched at j=4
            g0 = 2 * S * g
            xr2, xc2 = [], []
            for k in range(4):
                t = px.tile([128, 2 * S], BF, tag=f"xr{k}", name=f"xr{k}_{g}")
                nc.gpsimd.dma_start(t[:], xr_t[128 * k:128 * (k + 1), g0:g0 + 2 * S])
                xr2.append(t)
            for k in range(4):
                t = px.tile([128, 2 * S], BF, tag=f"xc{k}", name=f"xc{k}_{g}")
                nc.gpsimd.dma_start(t[:], xc_t[128 * k:128 * (k + 1), g0:g0 + 2 * S])
                xc2.append(t)
            return xr2, xc2

        xnext = load_x(0)

        # load order = consumption order: group-0 x tiles are queued first
        # (see load_x below); the first emitted matmuls are the h-branch QK
        # projections (whq, whkv), then v (wv), then attention constants,
        # then the MLP weights which are first needed one group later.
        whq = [load_const(f"whq{k}", w_hq[128 * k:128 * (k + 1), :], [128, E], BF)
               for k in range(4)]
        whkv = [load_const(f"whkv{k}", w_hkv[128 * k:128 * (k + 1), :], [128, 2 * E], BF)
                for k in range(4)]
        # wv split: QK columns land first (group-0 j-loop), V columns later
        wv = []
        for k in range(4):
            t = pw.tile([128, 3 * E], BF, tag=f"wv{k}")
            nc.sync.dma_start(t[:, 0:2 * E], w_vin[128 * k:128 * (k + 1), 0:2 * E])
            wv.append(t)
        for k in range(4):
            nc.sync.dma_start(wv[k][:, 2 * E:3 * E],
                              w_vin[128 * k:128 * (k + 1), 2 * E:3 * E])
        msk = load_const("msk", mask_d[:, :], [128, 384], BF)
        bia = load_const("bia", bias_d[:, :], [128, 24], F32)
        wfh = [load_const(f"wfh{k}", w_fh[128 * k:128 * (k + 1), :], [128, E], BF)
               for k in range(4)]
        wfv = [load_const(f"wfv{k}", w_fv[128 * k:128 * (k + 1), :], [128, E], BF)
               for k in range(4)]
        wm2 = [load_const(f"wm2{k}", w_m2[128 * k:128 * (k + 1), :], [128, E], BF)
               for k in range(4)]

        # bias cols: 0-7 v_in QK; 8-11 h Q; 12-15 h K; 16-19 b_eff; 20-23 b2

        prev_mlp = []   # deferred MLP closures from the previous group
        for g in range(NPAIR):
            g0 = 2 * S * g
            xr2, xc2 = xnext

            # ---------- QK projections, feature-major, N=384 ----------
            # emitted in score-consumption order: pair p's Q (j=p) and K
            # (j=4+p) for both branches land before pair p+1's, so pair-0
            # scores never wait on late evictions
            qk = {"h": [None] * 8, "v": [None] * 8}
            for p in range(4):
                for j in (p, 4 + p):
                    for br in ("h", "v"):
                        ps = psA.tile([128, 384], F32, tag="proj",
                                      padded_shape=[128, 512])
                        for k in range(4):
                            if br == "v":
                                lhsT = wv[k][:, 128 * j:128 * (j + 1)]
                                rhs = xr2[k][:]
                            elif j < 4:
                                lhsT = whq[k][:, 128 * j:128 * (j + 1)]
                                rhs = xr2[k][:]
                            else:
                                lhsT = whkv[k][:, 128 * (j - 4):128 * (j - 3)]
                                rhs = xc2[k][:]
                            nc.tensor.matmul(ps[:], lhsT=lhsT, rhs=rhs,
                                             start=(k == 0), stop=(k == 3))
                        bcol = j if br == "v" else (8 + j)
                        has_b = ((has_vqk_b and br == "v")
                                 or (has_hq_b and br == "h" and j < 4)
                                 or (has_hk_b and br == "h" and j >= 4))
                        dst = pqk.tile([128, 384], BF, tag="qk")
                        if has_b:
                            nc.scalar.activation(dst[:], ps[:], AF.Identity,
                                                 bias=bia[:, bcol:bcol + 1])
                        else:
                            # no bias: split evictions across engines by br
                            if br == "h":
                                nc.vector.tensor_copy(dst[:], ps[:])
                            else:
                                nc.scalar.activation(dst[:], ps[:], AF.Identity)
                        qk[br][j] = dst

            if g + 1 < NPAIR:
                xnext = load_x(g + 1)

            # ct: [128 f, 8 f-blocks (0-3 h, 4-7 v), 2S tokens] single tile
            # so the feature-major turn is 2 XBAR DMA-transposes per strip.
            # Final group: one tile per strip instead — DMA-transpose write
            # tracking is tile-granular, and the strip-0 MLP half must not
            # wait on strip-1's transposes.
            if g + 1 < NPAIR:
                ct = pct.tile([128, 8, 2 * S], BF, tag="ct", name=f"ct_{g}")
                cts = [ct]
            else:
                cts = [pct.tile([128, 8, S], BF, tag=f"ctf{a}", name=f"ctf_{a}")
                       for a in range(2)]
                ct = None

            for a in range(2):
                s0 = S * a
                # ---- V projections for both branches first ----
                vab = {}
                for br in ("h", "v"):
                    xin = xr2 if br == "v" else xc2
                    vcols = slice(1024, 1536) if br == "v" else slice(512, 1024)
                    vw = wv if br == "v" else whkv
                    vps_a = psA.tile([128, 512], F32, tag="proj")
                    for k in range(4):
                        nc.tensor.matmul(vps_a[:], lhsT=xin[k][:, s0:s0 + 128],
                                         rhs=vw[k][:, vcols],
                                         start=(k == 0), stop=(k == 3))
                    vps_b = psA.tile([128, 512], F32, tag="proj")
                    for k in range(4):
                        nc.tensor.matmul(vps_b[:], lhsT=xin[k][:, s0 + 64:s0 + 192],
                                         rhs=vw[k][:, vcols],
                                         start=(k == 0), stop=(k == 3))
                    va = pv.tile([128, 8, 65], BF, tag="vp")
                    vb = pv.tile([128, 8, 65], BF, tag="vp")
                    nc.vector.tensor_copy(
                        va[:, :, 0:64],
                        vps_a[:].rearrange("p (h c) -> p h c", c=64))
                    nc.vector.tensor_copy(
                        vb[:, :, 0:64],
                        vps_b[:].rearrange("p (h c) -> p h c", c=64))
                    if g == 0:
                        # ones columns: group 0's 8 allocs touch all 8 pv
                        # buffers once; later strips only rewrite [:, :, 0:64]
                        nc.vector.memset(va[:, :, 64:65], 1.0)
                        nc.vector.memset(vb[:, :, 64:65], 1.0)
                    vab[br] = (va, vb)

                # ---- scores + exp + mask, pair-steps interleaved h/v ----
                # pm[br][p]: (128, 448) masked probs for heads 2p, 2p+1
                pm = {"h": [None] * 4, "v": [None] * 4}
                # attn@V bank plan: per br, 3 psC tiles:
                #   A: heads 0-2, B: heads 3-5, C: heads 6-7
                cxt = {"h": [None] * 3, "v": [None] * 3}
                zrs = {"h": [None] * 3, "v": [None] * 3}
                # normalized ctx, token-major: [96 q', 2 (h|v), 512 f] so one
                # XBAR transpose covers both branches
                ctxn1 = pp.tile([96, 2, 512], BF, tag="ctxn1", bufs=4)
                ctxn2 = pp.tile([96, 2, 512], BF, tag="ctxn2", bufs=4)
                BI = {"h": 0, "v": 1}

                def emit_pair(br, p, spt=None):
                    # both branches of a pair-step share one [128,1024] tile:
                    # bank A (cols 0:512) holds the h2=0 blocks of h and v,
                    # bank B the h2=1 blocks, keeping each PSUM bank row-
                    # group pure (quad row-tiles drain to distinct banks)
                    boff = 0 if br == "h" else 192
                    QT = qk[br][p][:, s0:s0 + S]
                    KT = qk[br][4 + p][:, s0:s0 + S]
                    mmsc = []
                    for h2 in range(2):
                        d0 = 64 * h2
                        c0 = 512 * h2 + boff
                        mmsc.append(nc.tensor.matmul(
                            spt[:, c0:c0 + 96],
                            lhsT=KT[d0:d0 + 64, 0:128],
                            rhs=QT[d0:d0 + 64, 0:96],
                            start=True, stop=True, skip_group_check=True))
                        mmsc.append(nc.tensor.matmul(
                            spt[:, c0 + 96:c0 + 192],
                            lhsT=KT[d0:d0 + 64, 64:192],
                            rhs=QT[d0:d0 + 64, 96:192],
                            start=True, stop=True, skip_group_check=True))
                    pb = pp.tile([128, 384], BF, tag="p")
                    sin = spt[:].rearrange("p (b c) -> p b c", c=512)[:, :, boff:boff + 192]
                    ex = nc.scalar.activation(
                        pb[:].rearrange("p (b c) -> p b c", c=192),
                        sin, AF.Exp, scale=0.125)
                    for m in mmsc:
                        add_dep_helper(ex.ins, m.ins, sync=True,
                                       reason="exp after score mms")
                    pmt = pp.tile([128, 384], BF, tag="p")
                    nc.vector.tensor_tensor(pmt[:], pb[:], msk[:], op=MUL)
                    pm[br][p] = pmt

                def bank_mms(br, b):
                    """attn@V bank b: heads hs = 3b..3b+2 (bank 2: h6,h7).
                    Per head 2 matmuls: q' [0,96) from TA keys [0,128) (va),
                    q' [96,192) from TB keys [64,192) (vb). Output rows 0:96,
                    head i at cols [130i, 130i+130) = [q1' 65 | q2' 65]."""
                    hs = [3 * b + i for i in range(3 if b < 2 else 2)]
                    va, vb = vab[br]
                    cp = psC.tile([128, 130 * len(hs)], F32, tag="cx",
                                  padded_shape=[128, 512], name=f"cp_{br}_{b}")
                    mms = []
                    n = 2 * len(hs)
                    for i, h in enumerate(hs):
                        pmt = pm[br][h // 2]
                        ta = 192 * (h % 2)
                        cb = 130 * i
                        mms.append(lambda i=i, h=h, pmt=pmt, ta=ta, cb=cb: nc.tensor.matmul(
                            cp[0:96, cb:cb + 65], lhsT=pmt[:, ta:ta + 96],
                            rhs=va[:, h:h + 1, :], start=(2 * i == 0),
                            stop=(2 * i == n - 1), skip_group_check=True))
                        mms.append(lambda i=i, h=h, pmt=pmt, ta=ta, cb=cb: nc.tensor.matmul(
                            cp[0:96, cb + 65:cb + 130],
                            lhsT=pmt[:, ta + 96:ta + 192],
                            rhs=vb[:, h:h + 1, :], start=(2 * i + 1 == 0),
                            stop=(2 * i + 1 == n - 1), skip_group_check=True))
                    return cp, mms, hs

                def emit_banks(b):
                    """Emit h and v banks with matmuls interleaved so the
                    per-matmul SBUF latency of one bank's chain overlaps the
                    other bank's execution (different PSUM banks)."""
                    cph, mmh, hs = bank_mms("h", b)
                    cpv, mmv, _ = bank_mms("v", b)
                    outh, outv = [], []
                    for fh, fv in zip(mmh, mmv):
                        outh.append(fh())
                        outv.append(fv())
                    _chain(outh)
                    _chain(outv)
                    cxt["h"][b] = (cph, outh[-1], hs)
                    cxt["v"][b] = (cpv, outv[-1], hs)

                def emit_norm(br, b):
                    # normalize: one reciprocal + two broadcast tensor_tensor
                    # ops per bank ([96, nh, 64] x stride-0 recip) instead of
                    # 2*nh small per-head muls split across DVE/ScalarE
                    cp, lastmm, hs = cxt[br][b]
                    nh = len(hs)
                    h0 = hs[0]
                    zr = pzr.tile([96, 2, nh, 1], F32, tag="zr")
                    # Z columns viewed (qblock, head): [96, 2, nh, 1]
                    cpz = cp[0:96, 0:130 * nh].rearrange(
                        "p (h q c) -> p q h c", q=2, c=65)
                    reads = [nc.vector.reciprocal(zr[:], cpz[:, :, :, 64:65])]
                    cph = cp[0:96, 0:130 * nh].rearrange("p (x c) -> p x c", c=130)
                    for qb, dst in ((0, ctxn1), (1, ctxn2)):
                        in0 = cph[:, :, 65 * qb:65 * qb + 64]
                        b0, b1 = bass.broadcast_tensor_aps(in0, zr[:, qb, :, :])
                        out = dst[0:96, BI[br], 64 * h0:64 * (h0 + nh)]
                        reads.append(nc.vector.tensor_tensor(
                            out.rearrange("p (x c) -> p x c", c=64),
                            b0, b1, op=MUL))
                    for r in reads:
                        add_dep_helper(r.ins, lastmm.ins, sync=True,
                                       reason="psum read after group close")

                # emission: pair-steps with attn@V banks interleaved; the
                # previous group's MLP units are sprinkled between steps so
                # the in-order PE always has a ready big matmul to chew on
                # skip early sites: strip 0 waits out the previous group's
                # in-flight ct transposes; strip 1 defers so the last units
                # cover the late-strip banks2/norm2 stalls (12 units, 16
                # sites -> pops at strip-0 sites 2-8, strip-1 sites 4-8)
                skip = [1 if a == 0 else 3]

                def mlp_step():
                    if skip[0] > 0:
                        skip[0] -= 1
                        return
                    if prev_mlp:
                        prev_mlp.pop(0)()

                spt0 = psS.tile([128, 1024], F32, tag="sc")
                for br in ("h", "v"):
                    emit_pair(br, 0, spt0)
                mlp_step()
                spt1 = psS.tile([128, 1024], F32, tag="sc")
                for br in ("h", "v"):
                    emit_pair(br, 1, spt1)
                mlp_step()
                emit_banks(0)             # heads 0-2 (needs pairs 0,1)
                mlp_step()
                spt2 = psS.tile([128, 1024], F32, tag="sc")
                for br in ("h", "v"):
                    emit_pair(br, 2, spt2)
                mlp_step()
                for br in ("h", "v"):
                    emit_norm(br, 0)
                mlp_step()
                emit_banks(1)             # heads 3-5 (needs pairs 1,2)
                mlp_step()
                spt3 = psS.tile([128, 1024], F32, tag="sc")
                for br in ("h", "v"):
                    emit_pair(br, 3, spt3)
                mlp_step()
                for br in ("h", "v"):
                    emit_norm(br, 1)
                emit_banks(2)             # heads 6,7
                mlp_step()
                for br in ("h", "v"):
                    emit_norm(br, 2)

                # ---- feature-major turn via XBAR DMA transpose: one call
                # per (strip, q-block) [96,1024] -> [128, 8, 96]; the
                # ~1.3us cost per call sits on the idle sync queue ----
                ctd = cts[0] if g + 1 < NPAIR else cts[a]
                c0 = s0 if g + 1 < NPAIR else 0
                nc.sync.dma_start_transpose(
                    ctd[:, :, c0:c0 + 96], ctxn1[:, :, :])
                nc.sync.dma_start_transpose(
                    ctd[:, :, c0 + 96:c0 + 192], ctxn2[:, :, :])

            # ---------- fused out-proj + MLP1 + MLP2 as deferred closures,
            # emitted interleaved into the NEXT group's attention ----------
            def build_mlp(ct=cts[0], g0=g0):
                units = []
                hid = []

                psj = {}

                def hid_unit_a(j):
                    ps = psA.tile([128, 384], F32, tag="proj",
                                  padded_shape=[128, 512], name=f"mlp1_{j}")
                    psj[j] = ps
                    for k in range(4):
                        nc.tensor.matmul(ps[:],
                                         lhsT=wfh[k][:, 128 * j:128 * (j + 1)],
                                         rhs=ct[:, k, :],
                                         start=(k == 0), stop=False)

                def hid_unit_b(j):
                    ps = psj[j]
                    for k in range(4):
                        nc.tensor.matmul(ps[:],
                                         lhsT=wfv[k][:, 128 * j:128 * (j + 1)],
                                         rhs=ct[:, 4 + k, :],
                                         start=False, stop=(k == 3))
                    dst = phid.tile([128, 384], BF, tag="hid", name=f"hid_{j}")
                    if has_beff:
                        nc.scalar.activation(dst[:], ps[:], AF.Relu,
                                             bias=bia[:, 16 + j:16 + j + 1])
                    else:
                        nc.scalar.activation(dst[:], ps[:], AF.Relu)
                    hid.append(dst)

                def out_unit(j):
                    ps = psA.tile([128, 384], F32, tag="proj",
                                  padded_shape=[128, 512], name=f"mlp2_{j}")
                    for k in range(4):
                        nc.tensor.matmul(ps[:],
                                         lhsT=wm2[k][:, 128 * j:128 * (j + 1)],
                                         rhs=hid[k][:],
                                         start=(k == 0), stop=(k == 3))
                    osb = pout.tile([128, 384], BF, tag="o", name=f"osb_{j}")
                    if has_b2:
                        nc.scalar.activation(osb[:], ps[:], AF.Identity,
                                             bias=bia[:, 20 + j:20 + j + 1])
                    else:
                        nc.scalar.activation(osb[:], ps[:], AF.Identity)
                    nc.sync.dma_start(
                        out_t[128 * j:128 * (j + 1), g0:g0 + 2 * S], osb[:])

                for j in range(4):
                    units.append(lambda j=j: hid_unit_a(j))
                    units.append(lambda j=j: hid_unit_b(j))
                for j in range(4):
                    units.append(lambda j=j: out_unit(j))
                return units

            def build_mlp_final(cts=cts, g0=g0):
                """Final group: token columns are independent through the MLP
                (contraction is over features), so strip-0's half runs as
                self-contained units during strip-1's attention; only the
                strip-1 half waits on the last ct transposes."""
                hid = {}

                def get_hid(j):
                    if j not in hid:
                        hid[j] = phid.tile([128, 384], BF, tag="hid",
                                           name=f"hidF_{j}")
                    return hid[j]

                def full_unit(j, a):
                    cta = cts[a]
                    c0 = S * a
                    ps = psA.tile([128, 384], F32, tag="proj",
                                  padded_shape=[128, 512], name=f"mlpF_{j}_{a}")
                    for k in range(4):
                        nc.tensor.matmul(ps[:, 0:S],
                                         lhsT=wfh[k][:, 128 * j:128 * (j + 1)],
                                         rhs=cta[:, k, :],
                                         start=(k == 0), stop=False)
                    for k in range(4):
                        nc.tensor.matmul(ps[:, 0:S],
                                         lhsT=wfv[k][:, 128 * j:128 * (j + 1)],
                                         rhs=cta[:, 4 + k, :],
                                         start=False, stop=(k == 3))
                    dst = get_hid(j)
                    if has_beff:
                        nc.scalar.activation(dst[:, c0:c0 + S], ps[:, 0:S],
                                             AF.Relu,
                                             bias=bia[:, 16 + j:16 + j + 1])
                    else:
                        nc.scalar.activation(dst[:, c0:c0 + S], ps[:, 0:S],
                                             AF.Relu)

                def out_unit(j):
                    ps = psA.tile([128, 384], F32, tag="proj",
                                  padded_shape=[128, 512], name=f"mlp2F_{j}")
                    for k in range(4):
                        nc.tensor.matmul(ps[:],
                                         lhsT=wm2[k][:, 128 * j:128 * (j + 1)],
                                         rhs=get_hid(k)[:],
                                         start=(k == 0), stop=(k == 3))
                    osb = pout.tile([128, 384], BF, tag="o", name=f"osbF_{j}")
                    if has_b2:
                        nc.scalar.activation(osb[:], ps[:], AF.Identity,
                                             bias=bia[:, 20 + j:20 + j + 1])
                    else:
                        nc.scalar.activation(osb[:], ps[:], AF.Identity)
                    nc.sync.dma_start(
                        out_t[128 * j:128 * (j + 1), g0:g0 + 2 * S], osb[:])

                u0 = [lambda j=j: full_unit(j, 0) for j in range(4)]
                u1 = [lambda j=j: full_unit(j, 1) for j in range(4)]
                u1 += [lambda j=j: out_unit(j) for j in range(4)]
                return u0, u1

            for f in prev_mlp:   # drain any leftovers (shouldn't happen)
                f()
            if g + 1 < NPAIR:
                prev_mlp = build_mlp()
            else:
                fin_u0, fin_u1 = build_mlp_final()
                prev_mlp = fin_u0 + fin_u1

        for f in prev_mlp:       # final group's MLP
            f()
    nc.finalize()
    return nc


_CACHE = {}


def _get_program(bias_flags):
    key = tuple(bias_flags)
    if key not in _CACHE:
        _CACHE[key] = _build_program(key)
    return _CACHE[key]


def _col(b):
    return np.ascontiguousarray(b.reshape(-1, 128).T.astype(np.float32))


def kernel(hidden_states, h_in_w, h_in_b, h_out_w, h_out_b,
           v_in_w, v_in_b, v_out_w, v_out_b,
           mlp_w1, mlp_b1, mlp_w2, mlp_b2):
    x = np.asarray(hidden_states, dtype=np.float32)
    h_in_w = np.asarray(h_in_w, np.float32)
    h_in_b = np.asarray(h_in_b, np.float32)
    h_out_w = np.asarray(h_out_w, np.float32)
    h_out_b = np.asarray(h_out_b, np.float32)
    v_in_w = np.asarray(v_in_w, np.float32)
    v_in_b = np.asarray(v_in_b, np.float32)
    v_out_w = np.asarray(v_out_w, np.float32)
    v_out_b = np.asarray(v_out_b, np.float32)
    mlp_w1 = np.asarray(mlp_w1, np.float32)
    mlp_b1 = np.asarray(mlp_b1, np.float32)
    mlp_w2 = np.asarray(mlp_w2, np.float32)
    mlp_b2 = np.asarray(mlp_b2, np.float32)

    # V biases shift ctx by a constant (softmax weights sum to 1): fold through
    # out-proj; then fold out-proj entirely into MLP1 (relu is the only
    # nonlinearity after it): hid = relu(h_ctx@Wfh^T + v_ctx@Wfv^T + b_eff).
    h_out_eff = h_out_b + h_out_w @ h_in_b[2 * E:3 * E]
    v_out_eff = v_out_b + v_out_w @ v_in_b[2 * E:3 * E]
    W1h = mlp_w1[:, 0:E]
    W1v = mlp_w1[:, E:2 * E]
    Wfh = W1h @ h_out_w            # (E, E)
    Wfv = W1v @ v_out_w
    b_eff = mlp_b1 + W1h @ h_out_eff + W1v @ v_out_eff

    bias_flags = (
        bool(np.any(v_in_b[0:2 * E])), bool(np.any(h_in_b[0:E])),
        bool(np.any(h_in_b[E:2 * E])), bool(np.any(b_eff)),
        bool(np.any(mlp_b2)),
    )
    nc = _get_program(bias_flags)

    biases = np.zeros((128, 24), np.float32)
    biases[:, 0:8] = _col(v_in_b[0:2 * E])
    biases[:, 8:16] = _col(h_in_b[0:2 * E])
    biases[:, 16:20] = _col(b_eff)
    biases[:, 20:24] = _col(mlp_b2)

    shared = {
        "w_vin": np.ascontiguousarray(v_in_w.T).astype(NPBF),
        "w_hq": np.ascontiguousarray(h_in_w[0:E].T).astype(NPBF),
        "w_hkv": np.ascontiguousarray(h_in_w[E:3 * E].T).astype(NPBF),
        "w_fh": np.ascontiguousarray(Wfh.T).astype(NPBF),
        "w_fv": np.ascontiguousarray(Wfv.T).astype(NPBF),
        "w_m2": np.ascontiguousarray(mlp_w2.T).astype(NPBF),
        "mask": _band_masks(),
        "biases": biases,
    }

    in_maps = []
    for c in range(NCORE):
        rows = x[RPC * c:RPC * (c + 1)]
        cols = x[:, RPC * c:RPC * (c + 1)].transpose(1, 0, 2)
        m = dict(shared)
        m["xr_t"] = np.ascontiguousarray(rows.reshape(T, E).T).astype(NPBF)
        m["xc_t"] = np.ascontiguousarray(cols.reshape(T, E).T).astype(NPBF)
        in_maps.append(m)

    global _LAST_IN_MAPS
    _LAST_IN_MAPS = in_maps
    res = run_bass_kernel_spmd(nc, in_maps, core_ids=list(range(NCORE)))

    out = np.empty((S, S, E), np.float32)
    for c in range(NCORE):
        out[RPC * c:RPC * (c + 1)] = res.results[c]["out_t"].astype(np.float32).T.reshape(RPC, S, E)
    return out

